# revision 44
# baseline (speedup 1.0000x reference)
"""Grouped-query attention (GQA) Trainium2 Bass kernel, v2.

Problem: B=2, S=2048, DIM=2048, HQ=32, HKV=8, HEAD_DIM=64, causal mask.
Sharding: 8 cores = 2 (batch) x 4 (kv-head groups). Core c handles batch
c//4 and kv-block c%4 (2 kv heads, 8 q heads). Wq/Wk/Wv sharded
column-wise, Wo row-wise; each core writes a partial [S, DIM] bf16
output; host sums the 4 partials per batch and adds bo.

v2 changes vs the previous kernel (all bf16; fp8 fails the 2e-2 gate):
  - Host pre-transposes q/k/v (x^T tiles streamed as plain wide DMAs;
    no XBAR dma transpose -> faster first-tile arrival, cheaper DMA).
  - exp emitted over [128, 1024] PSUM pairs where both j-blocks are
    full width (fewer ACT instructions; ACT paces the score pipeline).
  - GEMM3 restructured: attn accumulated in natural [i, c] layout with
    exp blocks as the stationary operand and v(+ones) moving -> 65-row
    matmuls at full PE efficiency (~half the PE cycles of the j-layout),
    denominator lands as column 64 per i-partition.
  - Normalization fused into the PSUM->SBUF copy: DVE fp32 reciprocal
    of the denominator column + per-partition tensor_scalar multiply.
    The PE broadcast-matmul normalize of v1 is gone.
  - attn^T for GEMM4 via PE transpose of the normalized [128, 64]
    chunk; GPSIMD (Pool) copies the transposed chunk back to SBUF.
  - GEMM1 bias-adds moved from ACT to Pool so ACT does exp only.
  - GEMM4 output copies split DVE/Pool.
"""

import numpy as np
import ml_dtypes

import concourse.bass as bass
import concourse.mybir as mybir
from concourse import bacc
from concourse.tile import TileContext
from concourse.bass_utils import run_bass_kernel_spmd

F32 = mybir.dt.float32
BF16 = mybir.dt.bfloat16
AF = mybir.ActivationFunctionType
ALU = mybir.AluOpType

B, S, DIM = 2, 2048, 2048
HQ, HKV, HD = 32, 8, 64
GROUP = HQ // HKV              # 4
NCORES = 8
KVSH = 4                       # kv-blocks (shards) per batch
CQ = (HQ // KVSH) * HD         # 512 q-proj cols per core (8 heads)
CK = (HKV // KVSH) * HD        # 128 kv-proj cols per core (2 heads)
NDC = DIM // 128               # 16 contraction chunks
NSS = S // 512                 # 4 sequence chunks of 512


def build_nc2():
    """Causal-mode v2 builder."""
    nc = bacc.Bacc("TRN2", target_bir_lowering=False)

    # xt[p, ss*8192 + dc*512 + si] = x[ss*512+si, dc*128+p]
    qt = nc.dram_tensor("qt", [128, NDC * S], BF16, kind="ExternalInput")
    kt = nc.dram_tensor("kt", [128, NDC * S], BF16, kind="ExternalInput")
    vt = nc.dram_tensor("vt", [128, NDC * S], BF16, kind="ExternalInput")
    wq = nc.dram_tensor("wq", [128, NDC * CQ], BF16, kind="ExternalInput")
    wkv = nc.dram_tensor("wkv", [128, 2 * NDC * CK], BF16,
                         kind="ExternalInput")
    wo = nc.dram_tensor("wo", [128, 4 * DIM], BF16, kind="ExternalInput")
    bq = nc.dram_tensor("bq", [CQ], F32, kind="ExternalInput")
    bk = nc.dram_tensor("bk", [CK], F32, kind="ExternalInput")
    bv = nc.dram_tensor("bv", [CK], F32, kind="ExternalInput")
    tri = nc.dram_tensor("tri", [128, 128], BF16, kind="ExternalInput")
    ident = nc.dram_tensor("ident", [128, 128], BF16, kind="ExternalInput")
    out = nc.dram_tensor("out", [S, DIM], BF16, kind="ExternalOutput")

    XTS = {"q": qt, "k": kt, "v": vt}

    with TileContext(nc) as tc:
        with (
            tc.tile_pool(name="consts", bufs=1) as consts,
            tc.tile_pool(name="w", bufs=1) as wpool,
            tc.tile_pool(name="xt", bufs=1) as xt,
            tc.tile_pool(name="acts", bufs=1) as acts,
            tc.tile_pool(name="at2", bufs=2) as at2,
            tc.tile_pool(name="exp", bufs=18) as expp,
            tc.tile_pool(name="exps", bufs=14) as expsp,
            tc.tile_pool(name="nrm", bufs=6) as nrmp,
            tc.tile_pool(name="ob", bufs=5) as obp,
            tc.tile_pool(name="psc", bufs=2, space="PSUM") as psc,
            tc.tile_pool(name="psg", bufs=2, space="PSUM") as psg,
            tc.tile_pool(name="psx", bufs=2, space="PSUM") as psx,
        ):
            cn = {}

            def load_consts_early():
                cn["id"] = consts.tile([128, 128], BF16, tag="id",
                                       name="id_c")
                nc.sync.dma_start(out=cn["id"][:, :], in_=ident[:, :])
                cn["bk"] = consts.tile([128, 1], F32, tag="bk", name="bk_c")
                nc.sync.dma_start(
                    out=cn["bk"][:, :],
                    in_=bass.AP(tensor=bk[0:1].tensor, offset=0,
                                ap=[[1, 128], [128, 1]]))

            def load_consts():
                cn["tri"] = consts.tile([128, 128], BF16, tag="tri",
                                        name="tri_c")
                nc.sync.dma_start(out=cn["tri"][:, :], in_=tri[:, :])
                cn["bq"] = consts.tile([128, 4], F32, tag="bq", name="bq_c")
                nc.sync.dma_start(
                    out=cn["bq"][:, :],
                    in_=bass.AP(tensor=bq[0:1].tensor, offset=0,
                                ap=[[1, 128], [128, 4]]))
                cn["bv"] = consts.tile([128, 128], F32, tag="bv",
                                       name="bv_c")
                nc.sync.dma_start(
                    out=cn["bv"][:, :],
                    in_=bass.AP(tensor=bv[0:1].tensor, offset=0,
                                ap=[[0, 128], [1, 128]]))

            # ---- transposed input loads: plain DMA of host-packed x^T ----
            # Tiles cover [dc_lo, dc_hi) contraction chunks; finer tiles at
            # startup let the first GEMM1 chains begin sooner.
            HDC = NDC // 2
            XTB = {}

            def xtb_piece(ss, nm, dc_lo, dc_hi, tag):
                ndc = dc_hi - dc_lo
                t = xt.tile([128, ndc * 512], BF16, tag=tag,
                            name=f"x{tag}")
                c0 = ss * 8192 + dc_lo * 512
                nc.sync.dma_start(out=t[:, :],
                                  in_=XTS[nm][:, c0:c0 + ndc * 512])
                XTB.setdefault((ss, nm), []).append((dc_lo, dc_hi, t))

            def xtb_half(ss, nm, half):
                xtb_piece(ss, nm, half * HDC, (half + 1) * HDC,
                          f"x{nm}{half}")

            def xtb_load_t(ss, nm):
                xtb_half(ss, nm, 0)
                xtb_half(ss, nm, 1)

            def xtb_load(ss):
                for nm in "kvq":
                    xtb_load_t(ss, nm)

            def xslice(ss, nm, dc):
                for dc_lo, dc_hi, t in XTB[(ss, nm)]:
                    if dc_lo <= dc < dc_hi:
                        return t[:, (dc - dc_lo) * 512:(dc - dc_lo + 1) * 512]
                raise KeyError((ss, nm, dc))

            # ---- weights ----
            # prologue DMA order: k-chain first, then v (vx1 needed by the
            # first gemm3), then q per-cc chunks (cc-major wq layout).
            wk_bf = wpool.tile([128, NDC * CK], BF16, tag="wk", name="wk_bf")
            nc.sync.dma_start(out=wk_bf[:, :], in_=wkv[:, 0:2048])
            load_consts_early()
            for qt_ in range(4):
                xtb_piece(0, "k", qt_ * 4, (qt_ + 1) * 4, f"xk0q{qt_}")
            wv_bf = wpool.tile([128, NDC * CK], BF16, tag="wv", name="wv_bf")
            nc.sync.dma_start(out=wv_bf[:, :], in_=wkv[:, 2048:4096])
            xtb_load_t(0, "v")
            load_consts()
            xtb_load_t(0, "q")
            wqc = []
            for cc in range(4):
                wq_c = wpool.tile([128, NDC * 128], BF16, tag=f"wq{cc}",
                                  name=f"wq{cc}")
                nc.sync.dma_start(out=wq_c[:, :],
                                  in_=wq[:, cc * 2048:(cc + 1) * 2048])
                wqc.append(wq_c)
            wo_bf = wpool.tile([128, 4 * DIM], BF16, tag="wo", name="wo_bf")
            nc.sync.dma_start(out=wo_bf[:, :], in_=wo[:, :])

            # ---- persistent activations ----
            qxT = [acts.tile([128, S], BF16, tag=f"qx{cc}", name=f"qx{cc}")
                   for cc in range(4)]
            kxT = acts.tile([128, S], BF16, tag="kx", name="kx")
            vxT = acts.tile([128, S], BF16, tag="vx", name="vx")
            vx1 = [acts.tile([128, 130], BF16, tag=f"vp{sc}", name=f"vp{sc}")
                   for sc in range(S // 128)]

            def attnT(ss, cc):
                # double-buffered across ss (gemm4 runs one block behind)
                return at2.tile([128, 512], BF16, tag=f"at{cc}",
                                name=f"at{ss}{cc}")

            attnTs = {}

            def gemm1_q_cc(ss, cc):
                s0 = ss * 512
                ps = psx.tile([128, 512], F32, tag="m")
                for dc in range(NDC):
                    nc.tensor.matmul(
                        ps[:, :],
                        wqc[cc][:, dc * 128:(dc + 1) * 128],
                        xslice(ss, "q", dc),
                        start=(dc == 0), stop=(dc == NDC - 1))
                nc.vector.tensor_scalar_add(qxT[cc][:, s0:s0 + 512],
                                            ps[:, :], cn["bq"][:, cc:cc + 1])

            def gemm1_k(ss):
                s0 = ss * 512
                ps = psx.tile([128, 512], F32, tag="m")
                for dc in range(NDC):
                    nc.tensor.matmul(
                        ps[:, :], wk_bf[:, dc * 128:(dc + 1) * 128],
                        xslice(ss, "k", dc),
                        start=(dc == 0), stop=(dc == NDC - 1))
                nc.vector.tensor_scalar_add(kxT[:, s0:s0 + 512], ps[:, :],
                                            cn["bk"][:, 0:1])

            def gemm1_v(ss):
                s0 = ss * 512
                ps = psx.tile([128, 512], F32, tag="m")
                for dc in range(NDC):
                    nc.tensor.matmul(
                        ps[:, :], wv_bf[:, dc * 128:(dc + 1) * 128],
                        xslice(ss, "v", dc),
                        start=(dc == 0), stop=(dc == NDC - 1))
                nc.vector.tensor_copy(vxT[:, s0:s0 + 512], ps[:, :])

            def vtrans(ss):
                s0 = ss * 512
                vtp = psx.tile([128, 512], BF16, tag="m")
                for sc in range(4):
                    nc.tensor.transpose(
                        vtp[:, sc * 128:(sc + 1) * 128],
                        vxT[:, s0 + sc * 128:s0 + (sc + 1) * 128],
                        cn["id"][:, :])
                for sc in range(4):
                    jb = ss * 4 + sc
                    vx = vx1[jb]
                    for h2 in range(2):
                        nc.vector.tensor_tensor(
                            vx[:, h2 * 65:h2 * 65 + 64],
                            vtp[:, sc * 128 + h2 * 64:sc * 128 + (h2 + 1) * 64],
                            cn["bv"][:, h2 * 64:(h2 + 1) * 64], ALU.add)
                    nc.vector.memset(vx[:, 64:65], 1.0)
                    nc.vector.memset(vx[:, 129:130], 1.0)

            # exinfo[(ss, h, jb)] = (sbuf exp tile, col0, off)
            exinfo = {}

            def scores_grp(ss, h, jbs):
                """One PSUM pair-tile holding the given 1-2 j-blocks:
                matmuls + single exp (+ tri for diagonal blocks)."""
                s0 = ss * 512
                th, po, kv = h % 4, (h // GROUP) * 64, h // GROUP
                sp = psc.tile([128, 1024], F32, tag="sc")
                if len(jbs) == 2:
                    ex = expp.tile([128, 1024], BF16, tag="exp")
                else:
                    ex = expsp.tile([128, 512], BF16, tag="exps")
                tot = 0
                for t, jb in enumerate(jbs):
                    j0 = jb * 128
                    off = max(0, j0 - s0)
                    N = 512 - off
                    nc.tensor.matmul(
                        sp[:, t * 512:t * 512 + N],
                        kxT[kv * 64:(kv + 1) * 64, j0:j0 + 128],
                        qxT[th][po:po + 64, s0 + off:s0 + 512],
                        start=True, stop=True)
                    exinfo[(ss, h, jb)] = (ex, t * 512, off)
                    tot = t * 512 + N
                nc.scalar.activation(ex[:, :tot], sp[:, :tot], AF.Exp,
                                     scale=0.125)
                for t, jb in enumerate(jbs):
                    if jb >= 4 * ss:
                        c0 = t * 512
                        nc.gpsimd.tensor_tensor(
                            ex[:, c0:c0 + 128], ex[:, c0:c0 + 128],
                            cn["tri"][:, :], ALU.mult)

            def scores_plan(ss):
                """[(jb,), (jb, jb+1), ...] full blocks paired."""
                njb = 4 * (ss + 1)
                nfull = 4 * ss + 1
                grps = [(jb, jb + 1) for jb in range(0, nfull - 1, 2)]
                if nfull % 2 == 1:
                    grps.append((nfull - 1,))
                grps += [(jb,) for jb in range(nfull, njb)]
                return grps

            def gemm3_chunk(ss, h, sc):
                """attn chunk [128 i, 65] for i-block ib = 4ss+sc; returns
                psum tile."""
                kv = h // GROUP
                ib = 4 * ss + sc
                at = psg.tile([128, 512], F32, tag="g3")
                for jb in range(ib + 1):
                    ex, c0, off = exinfo[(ss, h, jb)]
                    nc.tensor.matmul(
                        at[:, 0:65],
                        ex[:, c0 + sc * 128 - off:c0 + sc * 128 - off + 128],
                        vx1[jb][:, kv * 65:kv * 65 + 65],
                        start=(jb == 0), stop=(jb == ib))
                return at

            def gemm3_norm(ss, h, sc, at):
                """fp32 reciprocal + fused normalize into SBUF copy."""
                rcp = nrmp.tile([128, 1], F32, tag="rcp")
                nc.vector.reciprocal(rcp[:, :], at[:, 64:65])
                an = nrmp.tile([128, 64], BF16, tag="an")
                nc.vector.tensor_scalar_mul(an[:, :], at[:, 0:64], rcp[:, :])
                return an

            def gemm3_ops(ss, h):
                """5 closures: chunk+norm x4 with transposes delayed so
                the DVE norm is long done, then one [64, 512] copy into
                attnT[ss]."""
                th, po = h % 4, (h // GROUP) * 64
                st = {}

                def chunk(sc):
                    def f():
                        at = gemm3_chunk(ss, h, sc)
                        st[sc] = gemm3_norm(ss, h, sc, at)
                        if sc == 2:
                            st["tr"] = psx.tile([64, 512], BF16, tag="m",
                                                name="tr")
                            for lo in (0, 1):
                                nc.tensor.transpose(
                                    st["tr"][0:64, lo * 128:(lo + 1) * 128],
                                    st[lo][:, :], cn["id"][:, :])
                        elif sc == 3:
                            nc.tensor.transpose(
                                st["tr"][0:64, 256:384],
                                st[2][:, :], cn["id"][:, :])
                    return f

                def fin():
                    nc.tensor.transpose(st["tr"][0:64, 384:512],
                                        st[3][:, :], cn["id"][:, :])
                    nc.vector.tensor_copy(
                        attnTs[(ss, th)][po:po + 64, :], st["tr"][0:64, :])

                return [chunk(0), chunk(1), chunk(2), chunk(3), fin]

            def gemm4_piece(ss, sc, eh):
                s0 = ss * 512
                i0 = s0 + sc * 128
                ob = obp.tile([128, 1024], BF16, tag="ob")
                for e2 in range(2):
                    ec = eh * 2 + e2
                    g4 = psx.tile([128, 512], F32, tag="m")
                    for cc2 in range(4):
                        nc.tensor.matmul(
                            g4[:, :],
                            attnTs[(ss, cc2)][:, sc * 128:(sc + 1) * 128],
                            wo_bf[:, cc2 * 2048 + ec * 512:
                                  cc2 * 2048 + (ec + 1) * 512],
                            start=(cc2 == 0), stop=(cc2 == 3))
                    nc.vector.tensor_copy(
                        ob[:, e2 * 512:(e2 + 1) * 512], g4[:, :])
                nc.sync.dma_start(
                    out=out[i0:i0 + 128, eh * 1024:(eh + 1) * 1024],
                    in_=ob[:, :])

            def block(ss, extra, carry_in):
                """scores(h) + gemm3(h-1) pipeline; carry_in = (pss, 7) of
                the previous block's last head, processed at h==0."""
                for cc in range(4):
                    attnTs[(ss, cc)] = attnT(ss, cc)
                if ss == 0:
                    gemm1_k(ss)
                    gemm1_v(ss)
                    vtrans(ss)
                    gemm1_q_cc(ss, 0)
                else:
                    gemm1_k(ss)
                    gemm1_v(ss)
                grps = scores_plan(ss)
                pending = list(carry_in or [])
                for h in range(8):
                    if h == 0 and ss > 0:
                        gemm1_q_cc(ss, 0)
                        vtrans(ss)
                        gemm1_q_cc(ss, 1)
                    elif h == 0:
                        gemm1_q_cc(ss, 1)
                    elif h == 1:
                        gemm1_q_cc(ss, 2)
                    elif h == 2:
                        gemm1_q_cc(ss, 3)
                    # interleave: scores groups of head h with gemm3 of the
                    # head TWO slots back (extra ACT slack) and this slot's
                    # extra ops (gemm4 etc.), fills front-loaded.
                    ng = len(grps)
                    g3ops = []
                    if len(pending) >= 2:
                        g3ops = gemm3_ops(*pending.pop(0))
                    fill = list(g3ops) + list(extra.get(h, ()))
                    n_emit, n_tot = 0, len(fill)
                    for gi, grp in enumerate(grps):
                        want = n_tot * gi // ng
                        while n_emit < want:
                            fill[n_emit]()
                            n_emit += 1
                        scores_grp(ss, h, grp)
                    while n_emit < n_tot:
                        fill[n_emit]()
                        n_emit += 1
                    pending.append((ss, h))
                return pending

            # ---- schedule ----
            xtb_load(1)
            carry = block(0, {}, None)

            ext1 = {h: [lambda h=h: gemm4_piece(0, (h - 1) // 2,
                                                (h - 1) % 2)]
                    for h in range(1, 8)}
            ext1[3].append(lambda: xtb_load_t(2, "k"))
            ext1[4].append(lambda: xtb_load_t(2, "v"))
            ext1[5].append(lambda: xtb_load_t(2, "q"))
            carry = block(1, ext1, carry)
            gemm4_piece(0, 3, 1)

            ext2 = {h: [lambda h=h: gemm4_piece(1, (h - 1) // 2,
                                                (h - 1) % 2)]
                    for h in range(1, 8)}
            ext2[3].append(lambda: xtb_load_t(3, "k"))
            ext2[4].append(lambda: xtb_load_t(3, "v"))
            ext2[5].append(lambda: xtb_load_t(3, "q"))
            carry = block(2, ext2, carry)
            gemm4_piece(1, 3, 1)

            ext3 = {h: [lambda h=h: gemm4_piece(2, (h - 1) // 2,
                                                (h - 1) % 2)]
                    for h in range(1, 8)}
            carry = block(3, ext3, carry)
            gemm4_piece(2, 3, 1)
            # drain: the last two heads' gemm3, then block 3's gemm4
            for pend in carry:
                for f in gemm3_ops(*pend):
                    f()
            for sc in range(4):
                gemm4_piece(3, sc, 0)
                gemm4_piece(3, sc, 1)
    nc.finalize()
    return nc


# ---------------- legacy (dense/no-mask) builder, unchanged ----------------

def build_nc(mode="causal"):
    if mode == "causal":
        return build_nc2()
    raise NotImplementedError("v2 kernel supports the causal mask only")


_CACHE = {}


def _get_nc(mode):
    if mode not in _CACHE:
        _CACHE[mode] = build_nc2() if mode == "causal" else None
    return _CACHE[mode]


def _host_xt(x, bf):
    # xt[p, ss*8192 + dc*512 + si] = x[ss*512+si, dc*128+p]
    xr = np.asarray(x, np.float32).reshape(NSS, 512, NDC, 128)
    return np.ascontiguousarray(
        xr.transpose(3, 0, 2, 1).reshape(128, NDC * S).astype(bf))


def kernel(q, k, v, mask, Wq, bq, Wk, bk, Wv, bv, Wo, bo):
    q = np.asarray(q, np.float32)
    k = np.asarray(k, np.float32)
    v = np.asarray(v, np.float32)
    mask = np.asarray(mask)
    Wq = np.asarray(Wq, np.float32)
    Wk = np.asarray(Wk, np.float32)
    Wv = np.asarray(Wv, np.float32)
    Wo = np.asarray(Wo, np.float32)
    bq = np.asarray(bq, np.float32)
    bk = np.asarray(bk, np.float32)
    bv = np.asarray(bv, np.float32)
    bo = np.asarray(bo, np.float32)

    m = mask.astype(np.float64)
    assert np.array_equal(m, np.tril(np.ones((S, S)))), \
        "v2 kernel supports the causal mask"

    nc = _get_nc("causal")
    bf = ml_dtypes.bfloat16
    tri_np = np.triu(np.ones((128, 128))).astype(bf)
    id_np = np.eye(128).astype(bf)

    head_perm = [h for cc in range(4) for h in (cc, cc + 4)]
    col_perm = np.concatenate(
        [np.arange(h * HD, (h + 1) * HD) for h in head_perm])

    in_maps = []
    for core in range(NCORES):
        b, kb = core // KVSH, core % KVSH
        wq_sh = Wq[:, kb * CQ:(kb + 1) * CQ][:, col_perm]
        wo_sh = Wo[kb * CQ:(kb + 1) * CQ, :][col_perm, :]
        bq_sh = bq[kb * CQ:(kb + 1) * CQ][col_perm]
        wk_sh = Wk[:, kb * CK:(kb + 1) * CK]
        wv_sh = Wv[:, kb * CK:(kb + 1) * CK]
        # cc-major: wq_arr[p, cc*2048 + dc*128 + j]
        wq_arr = wq_sh.reshape(NDC, 128, 4, 128).transpose(1, 2, 0, 3).reshape(
            128, NDC * CQ)
        wkv_arr = np.stack(
            [w.reshape(NDC, 128, CK).transpose(1, 0, 2).reshape(128, NDC * CK)
             for w in (wk_sh, wv_sh)], axis=1).reshape(128, 2 * NDC * CK)
        wo_arr = wo_sh.reshape(4, 128, DIM).transpose(1, 0, 2).reshape(
            128, 4 * DIM)
        im = {
            "qt": _host_xt(q[b], bf),
            "kt": _host_xt(k[b], bf),
            "vt": _host_xt(v[b], bf),
            "wq": np.ascontiguousarray(wq_arr.astype(bf)),
            "wkv": np.ascontiguousarray(wkv_arr.astype(bf)),
            "wo": np.ascontiguousarray(wo_arr.astype(bf)),
            "bq": np.ascontiguousarray(bq_sh),
            "bk": np.ascontiguousarray(bk[kb * CK:(kb + 1) * CK]),
            "bv": np.ascontiguousarray(bv[kb * CK:(kb + 1) * CK]),
            "tri": tri_np,
            "ident": id_np,
        }
        in_maps.append(im)

    res = run_bass_kernel_spmd(nc, in_maps, core_ids=list(range(NCORES)))
    outs = [r["out"] for r in res.results]
    full = np.empty((B, S, DIM), np.float32)
    for b in range(B):
        acc = outs[b * KVSH].astype(np.float32)
        for kb in range(1, KVSH):
            acc = acc + outs[b * KVSH + kb].astype(np.float32)
        full[b] = acc + bo[None, :]
    return full


# revision 47
# speedup vs baseline: 1.0143x; 1.0143x over previous
"""Grouped-query attention (GQA) Trainium2 Bass kernel, v2.

Problem: B=2, S=2048, DIM=2048, HQ=32, HKV=8, HEAD_DIM=64, causal mask.
Sharding: 8 cores = 2 (batch) x 4 (kv-head groups). Core c handles batch
c//4 and kv-block c%4 (2 kv heads, 8 q heads). Wq/Wk/Wv sharded
column-wise, Wo row-wise; each core writes a partial [S, DIM] bf16
output; host sums the 4 partials per batch and adds bo.

v2 changes vs the previous kernel (all bf16; fp8 fails the 2e-2 gate):
  - Host pre-transposes q/k/v (x^T tiles streamed as plain wide DMAs;
    no XBAR dma transpose -> faster first-tile arrival, cheaper DMA).
  - exp emitted over [128, 1024] PSUM pairs where both j-blocks are
    full width (fewer ACT instructions; ACT paces the score pipeline).
  - GEMM3 restructured: attn accumulated in natural [i, c] layout with
    exp blocks as the stationary operand and v(+ones) moving -> 65-row
    matmuls at full PE efficiency (~half the PE cycles of the j-layout),
    denominator lands as column 64 per i-partition.
  - Normalization fused into the PSUM->SBUF copy: DVE fp32 reciprocal
    of the denominator column + per-partition tensor_scalar multiply.
    The PE broadcast-matmul normalize of v1 is gone.
  - attn^T for GEMM4 via PE transpose of the normalized [128, 64]
    chunk; GPSIMD (Pool) copies the transposed chunk back to SBUF.
  - GEMM1 bias-adds moved from ACT to Pool so ACT does exp only.
  - GEMM4 output copies split DVE/Pool.
"""

import numpy as np
import ml_dtypes

import concourse.bass as bass
import concourse.mybir as mybir
from concourse import bacc
from concourse.tile import TileContext
from concourse.bass_utils import run_bass_kernel_spmd

F32 = mybir.dt.float32
BF16 = mybir.dt.bfloat16
AF = mybir.ActivationFunctionType
ALU = mybir.AluOpType

B, S, DIM = 2, 2048, 2048
HQ, HKV, HD = 32, 8, 64
GROUP = HQ // HKV              # 4
NCORES = 8
KVSH = 4                       # kv-blocks (shards) per batch
CQ = (HQ // KVSH) * HD         # 512 q-proj cols per core (8 heads)
CK = (HKV // KVSH) * HD        # 128 kv-proj cols per core (2 heads)
NDC = DIM // 128               # 16 contraction chunks
NSS = S // 512                 # 4 sequence chunks of 512


def build_nc2():
    """Causal-mode v2 builder."""
    nc = bacc.Bacc("TRN2", target_bir_lowering=False)

    # xt[p, ss*8192 + dc*512 + si] = x[ss*512+si, dc*128+p]
    qt = nc.dram_tensor("qt", [128, NDC * S], BF16, kind="ExternalInput")
    kt = nc.dram_tensor("kt", [128, NDC * S], BF16, kind="ExternalInput")
    vt = nc.dram_tensor("vt", [128, NDC * S], BF16, kind="ExternalInput")
    wq = nc.dram_tensor("wq", [128, NDC * CQ], BF16, kind="ExternalInput")
    wkv = nc.dram_tensor("wkv", [128, 2 * NDC * CK], BF16,
                         kind="ExternalInput")
    wo = nc.dram_tensor("wo", [128, 4 * DIM], BF16, kind="ExternalInput")
    bq = nc.dram_tensor("bq", [CQ], F32, kind="ExternalInput")
    bk = nc.dram_tensor("bk", [CK], F32, kind="ExternalInput")
    bv = nc.dram_tensor("bv", [CK], F32, kind="ExternalInput")
    tri = nc.dram_tensor("tri", [128, 128], BF16, kind="ExternalInput")
    ident = nc.dram_tensor("ident", [128, 128], BF16, kind="ExternalInput")
    out = nc.dram_tensor("out", [S, DIM], BF16, kind="ExternalOutput")

    XTS = {"q": qt, "k": kt, "v": vt}

    with TileContext(nc) as tc:
        with (
            tc.tile_pool(name="consts", bufs=1) as consts,
            tc.tile_pool(name="w", bufs=1) as wpool,
            tc.tile_pool(name="xt", bufs=1) as xt,
            tc.tile_pool(name="acts", bufs=1) as acts,
            tc.tile_pool(name="at2", bufs=2) as at2,
            tc.tile_pool(name="exp", bufs=21) as expp,
            tc.tile_pool(name="exps", bufs=4) as expsp,
            tc.tile_pool(name="nrm", bufs=6) as nrmp,
            tc.tile_pool(name="ob", bufs=5) as obp,
            tc.tile_pool(name="psc", bufs=2, space="PSUM") as psc,
            tc.tile_pool(name="psg", bufs=2, space="PSUM") as psg,
            tc.tile_pool(name="psx", bufs=2, space="PSUM") as psx,
        ):
            cn = {}

            def load_consts_early():
                cn["id"] = consts.tile([128, 128], BF16, tag="id",
                                       name="id_c")
                nc.sync.dma_start(out=cn["id"][:, :], in_=ident[:, :])
                cn["bk"] = consts.tile([128, 1], F32, tag="bk", name="bk_c")
                nc.sync.dma_start(
                    out=cn["bk"][:, :],
                    in_=bass.AP(tensor=bk[0:1].tensor, offset=0,
                                ap=[[1, 128], [128, 1]]))

            def load_consts():
                cn["tri"] = consts.tile([128, 128], BF16, tag="tri",
                                        name="tri_c")
                nc.sync.dma_start(out=cn["tri"][:, :], in_=tri[:, :])
                cn["bq"] = consts.tile([128, 4], F32, tag="bq", name="bq_c")
                nc.sync.dma_start(
                    out=cn["bq"][:, :],
                    in_=bass.AP(tensor=bq[0:1].tensor, offset=0,
                                ap=[[1, 128], [128, 4]]))
                cn["bv"] = consts.tile([128, 128], F32, tag="bv",
                                       name="bv_c")
                nc.sync.dma_start(
                    out=cn["bv"][:, :],
                    in_=bass.AP(tensor=bv[0:1].tensor, offset=0,
                                ap=[[0, 128], [1, 128]]))

            # ---- transposed input loads: plain DMA of host-packed x^T ----
            # Tiles cover [dc_lo, dc_hi) contraction chunks; finer tiles at
            # startup let the first GEMM1 chains begin sooner.
            HDC = NDC // 2
            XTB = {}

            def xtb_piece(ss, nm, dc_lo, dc_hi, tag):
                ndc = dc_hi - dc_lo
                t = xt.tile([128, ndc * 512], BF16, tag=tag,
                            name=f"x{tag}")
                c0 = ss * 8192 + dc_lo * 512
                nc.sync.dma_start(out=t[:, :],
                                  in_=XTS[nm][:, c0:c0 + ndc * 512])
                XTB.setdefault((ss, nm), []).append((dc_lo, dc_hi, t))

            def xtb_half(ss, nm, half):
                xtb_piece(ss, nm, half * HDC, (half + 1) * HDC,
                          f"x{nm}{half}")

            def xtb_load_t(ss, nm):
                xtb_half(ss, nm, 0)
                xtb_half(ss, nm, 1)

            def xtb_load(ss):
                for nm in "kvq":
                    xtb_load_t(ss, nm)

            def xslice(ss, nm, dc):
                for dc_lo, dc_hi, t in XTB[(ss, nm)]:
                    if dc_lo <= dc < dc_hi:
                        return t[:, (dc - dc_lo) * 512:(dc - dc_lo + 1) * 512]
                raise KeyError((ss, nm, dc))

            # ---- weights ----
            # prologue DMA order: k-chain first, then v (vx1 needed by the
            # first gemm3), then q per-cc chunks (cc-major wq layout).
            wk_bf = wpool.tile([128, NDC * CK], BF16, tag="wk", name="wk_bf")
            nc.sync.dma_start(out=wk_bf[:, :], in_=wkv[:, 0:2048])
            load_consts_early()
            for qt_ in range(4):
                xtb_piece(0, "k", qt_ * 4, (qt_ + 1) * 4, f"xk0q{qt_}")
            wv_bf = wpool.tile([128, NDC * CK], BF16, tag="wv", name="wv_bf")
            nc.sync.dma_start(out=wv_bf[:, :], in_=wkv[:, 2048:4096])
            xtb_load_t(0, "v")
            load_consts()
            wqc = []

            def load_wqc(cc):
                wq_c = wpool.tile([128, NDC * 128], BF16, tag=f"wq{cc}",
                                  name=f"wq{cc}")
                nc.sync.dma_start(out=wq_c[:, :],
                                  in_=wq[:, cc * 2048:(cc + 1) * 2048])
                wqc.append(wq_c)

            xtb_half(0, "q", 0)
            load_wqc(0)
            xtb_half(0, "q", 1)
            for cc in range(1, 4):
                load_wqc(cc)
            wo_bf = wpool.tile([128, 4 * DIM], BF16, tag="wo", name="wo_bf")
            nc.sync.dma_start(out=wo_bf[:, :], in_=wo[:, :])

            # ---- persistent activations ----
            qxT = [acts.tile([128, S], BF16, tag=f"qx{cc}", name=f"qx{cc}")
                   for cc in range(4)]
            kxT = acts.tile([128, S], BF16, tag="kx", name="kx")
            vxT = acts.tile([128, S], BF16, tag="vx", name="vx")
            vx1 = [acts.tile([128, 130], BF16, tag=f"vp{sc}", name=f"vp{sc}")
                   for sc in range(S // 128)]

            def attnT(ss, cc):
                # double-buffered across ss (gemm4 runs one block behind)
                return at2.tile([128, 512], BF16, tag=f"at{cc}",
                                name=f"at{ss}{cc}")

            attnTs = {}

            def gemm1_q_cc(ss, cc):
                s0 = ss * 512
                ps = psx.tile([128, 512], F32, tag="m")
                for dc in range(NDC):
                    nc.tensor.matmul(
                        ps[:, :],
                        wqc[cc][:, dc * 128:(dc + 1) * 128],
                        xslice(ss, "q", dc),
                        start=(dc == 0), stop=(dc == NDC - 1))
                nc.vector.tensor_scalar_add(qxT[cc][:, s0:s0 + 512],
                                            ps[:, :], cn["bq"][:, cc:cc + 1])

            def gemm1_k(ss):
                s0 = ss * 512
                ps = psx.tile([128, 512], F32, tag="m")
                for dc in range(NDC):
                    nc.tensor.matmul(
                        ps[:, :], wk_bf[:, dc * 128:(dc + 1) * 128],
                        xslice(ss, "k", dc),
                        start=(dc == 0), stop=(dc == NDC - 1))
                nc.vector.tensor_scalar_add(kxT[:, s0:s0 + 512], ps[:, :],
                                            cn["bk"][:, 0:1])

            def gemm1_v(ss):
                s0 = ss * 512
                ps = psx.tile([128, 512], F32, tag="m")
                for dc in range(NDC):
                    nc.tensor.matmul(
                        ps[:, :], wv_bf[:, dc * 128:(dc + 1) * 128],
                        xslice(ss, "v", dc),
                        start=(dc == 0), stop=(dc == NDC - 1))
                nc.vector.tensor_copy(vxT[:, s0:s0 + 512], ps[:, :])

            def vtrans(ss):
                s0 = ss * 512
                vtp = psx.tile([128, 512], BF16, tag="m")
                for sc in range(4):
                    nc.tensor.transpose(
                        vtp[:, sc * 128:(sc + 1) * 128],
                        vxT[:, s0 + sc * 128:s0 + (sc + 1) * 128],
                        cn["id"][:, :])
                for sc in range(4):
                    jb = ss * 4 + sc
                    vx = vx1[jb]
                    for h2 in range(2):
                        nc.vector.tensor_tensor(
                            vx[:, h2 * 65:h2 * 65 + 64],
                            vtp[:, sc * 128 + h2 * 64:sc * 128 + (h2 + 1) * 64],
                            cn["bv"][:, h2 * 64:(h2 + 1) * 64], ALU.add)
                    nc.vector.memset(vx[:, 64:65], 1.0)
                    nc.vector.memset(vx[:, 129:130], 1.0)

            # exinfo[(ss, h, jb)] = (sbuf exp tile, col0, off)
            exinfo = {}

            def scores_grp(ss, h, grp):
                """One PSUM tile holding the given (jb, col0) j-blocks
                packed back-to-back: matmuls + a single exp (+ tri for
                diagonal blocks)."""
                s0 = ss * 512
                th, po, kv = h % 4, (h // GROUP) * 64, h // GROUP
                tot = max(c0 + 512 - max(0, jb * 128 - s0)
                          for jb, c0 in grp)
                sp = psc.tile([128, 1024], F32, tag="sc")
                if tot > 512:
                    ex = expp.tile([128, 1024], BF16, tag="exp")
                else:
                    ex = expsp.tile([128, 512], BF16, tag="exps")
                for jb, c0 in grp:
                    j0 = jb * 128
                    off = max(0, j0 - s0)
                    N = 512 - off
                    nc.tensor.matmul(
                        sp[:, c0:c0 + N],
                        kxT[kv * 64:(kv + 1) * 64, j0:j0 + 128],
                        qxT[th][po:po + 64, s0 + off:s0 + 512],
                        start=True, stop=True)
                    exinfo[(ss, h, jb)] = (ex, c0, off)
                nc.scalar.activation(ex[:, :tot], sp[:, :tot], AF.Exp,
                                     scale=0.125)
                for jb, c0 in grp:
                    if jb >= 4 * ss:
                        nc.gpsimd.tensor_tensor(
                            ex[:, c0:c0 + 128], ex[:, c0:c0 + 128],
                            cn["tri"][:, :], ALU.mult)

            def scores_plan(ss):
                """Full blocks paired; the diagonal tail packed as
                [512|384|128] (exactly one [128,1024] tile) + [256]."""
                d = 4 * ss
                grps = [[(jb, 0), (jb + 1, 512)] for jb in range(0, d, 2)]
                grps.append([(d, 0), (d + 1, 512), (d + 3, 896)])
                grps.append([(d + 2, 0)])
                return grps

            def gemm3_chunk(ss, h, sc):
                """attn chunk [128 i, 65] for i-block ib = 4ss+sc; returns
                psum tile."""
                kv = h // GROUP
                ib = 4 * ss + sc
                at = psg.tile([128, 512], F32, tag="g3")
                for jb in range(ib + 1):
                    ex, c0, off = exinfo[(ss, h, jb)]
                    nc.tensor.matmul(
                        at[:, 0:65],
                        ex[:, c0 + sc * 128 - off:c0 + sc * 128 - off + 128],
                        vx1[jb][:, kv * 65:kv * 65 + 65],
                        start=(jb == 0), stop=(jb == ib))
                return at

            def gemm3_norm(ss, h, sc, at):
                """fp32 reciprocal + fused normalize into SBUF copy."""
                rcp = nrmp.tile([128, 1], F32, tag="rcp")
                nc.vector.reciprocal(rcp[:, :], at[:, 64:65])
                an = nrmp.tile([128, 64], BF16, tag="an")
                nc.vector.tensor_scalar_mul(an[:, :], at[:, 0:64], rcp[:, :])
                return an

            def gemm3_ops(ss, h):
                """5 closures: chunk+norm x4 with transposes delayed so
                the DVE norm is long done, then one [64, 512] copy into
                attnT[ss]."""
                th, po = h % 4, (h // GROUP) * 64
                st = {}

                def chunk(sc):
                    def f():
                        at = gemm3_chunk(ss, h, sc)
                        st[sc] = gemm3_norm(ss, h, sc, at)
                        if sc == 2:
                            st["tr"] = psx.tile([64, 512], BF16, tag="m",
                                                name="tr")
                            for lo in (0, 1):
                                nc.tensor.transpose(
                                    st["tr"][0:64, lo * 128:(lo + 1) * 128],
                                    st[lo][:, :], cn["id"][:, :])
                        elif sc == 3:
                            nc.tensor.transpose(
                                st["tr"][0:64, 256:384],
                                st[2][:, :], cn["id"][:, :])
                    return f

                def fin():
                    nc.tensor.transpose(st["tr"][0:64, 384:512],
                                        st[3][:, :], cn["id"][:, :])
                    nc.vector.tensor_copy(
                        attnTs[(ss, th)][po:po + 64, :], st["tr"][0:64, :])

                return [chunk(0), chunk(1), chunk(2), chunk(3), fin]

            def gemm4_piece(ss, sc, eh):
                s0 = ss * 512
                i0 = s0 + sc * 128
                ob = obp.tile([128, 1024], BF16, tag="ob")
                for e2 in range(2):
                    ec = eh * 2 + e2
                    g4 = psx.tile([128, 512], F32, tag="m")
                    for cc2 in range(4):
                        nc.tensor.matmul(
                            g4[:, :],
                            attnTs[(ss, cc2)][:, sc * 128:(sc + 1) * 128],
                            wo_bf[:, cc2 * 2048 + ec * 512:
                                  cc2 * 2048 + (ec + 1) * 512],
                            start=(cc2 == 0), stop=(cc2 == 3))
                    nc.vector.tensor_copy(
                        ob[:, e2 * 512:(e2 + 1) * 512], g4[:, :])
                nc.sync.dma_start(
                    out=out[i0:i0 + 128, eh * 1024:(eh + 1) * 1024],
                    in_=ob[:, :])

            def block(ss, extra, carry_in):
                """scores(h) + gemm3(h-1) pipeline; carry_in = (pss, 7) of
                the previous block's last head, processed at h==0."""
                for cc in range(4):
                    attnTs[(ss, cc)] = attnT(ss, cc)
                if ss == 0:
                    gemm1_k(ss)
                    gemm1_v(ss)
                    vtrans(ss)
                    gemm1_q_cc(ss, 0)
                else:
                    gemm1_k(ss)
                    gemm1_v(ss)
                grps = scores_plan(ss)
                pending = list(carry_in or [])
                for h in range(8):
                    if h == 0 and ss > 0:
                        gemm1_q_cc(ss, 0)
                        vtrans(ss)
                        gemm1_q_cc(ss, 1)
                    elif h == 0:
                        gemm1_q_cc(ss, 1)
                    elif h == 1:
                        gemm1_q_cc(ss, 2)
                    elif h == 2:
                        gemm1_q_cc(ss, 3)
                    # interleave: scores groups of head h with gemm3 of the
                    # head TWO slots back (extra ACT slack) and this slot's
                    # extra ops (gemm4 etc.), fills front-loaded.
                    ng = len(grps)
                    g3ops = []
                    if len(pending) >= 2:
                        g3ops = gemm3_ops(*pending.pop(0))
                    fill = list(g3ops) + list(extra.get(h, ()))
                    n_emit, n_tot = 0, len(fill)
                    for gi, grp in enumerate(grps):
                        want = n_tot * gi // ng
                        while n_emit < want:
                            fill[n_emit]()
                            n_emit += 1
                        scores_grp(ss, h, grp)
                    while n_emit < n_tot:
                        fill[n_emit]()
                        n_emit += 1
                    pending.append((ss, h))
                return pending

            # ---- schedule ----
            xtb_load(1)
            carry = block(0, {}, None)

            ext1 = {h: [lambda h=h: gemm4_piece(0, (h - 1) // 2,
                                                (h - 1) % 2)]
                    for h in range(1, 8)}
            ext1[3].append(lambda: xtb_load_t(2, "k"))
            ext1[4].append(lambda: xtb_load_t(2, "v"))
            ext1[5].append(lambda: xtb_load_t(2, "q"))
            carry = block(1, ext1, carry)
            gemm4_piece(0, 3, 1)

            ext2 = {h: [lambda h=h: gemm4_piece(1, (h - 1) // 2,
                                                (h - 1) % 2)]
                    for h in range(1, 8)}
            ext2[3].append(lambda: xtb_load_t(3, "k"))
            ext2[4].append(lambda: xtb_load_t(3, "v"))
            ext2[5].append(lambda: xtb_load_t(3, "q"))
            carry = block(2, ext2, carry)
            gemm4_piece(1, 3, 1)

            ext3 = {h: [lambda h=h: gemm4_piece(2, (h - 1) // 2,
                                                (h - 1) % 2)]
                    for h in range(1, 8)}
            carry = block(3, ext3, carry)
            gemm4_piece(2, 3, 1)
            # drain: the last two heads' gemm3, then block 3's gemm4
            for pend in carry:
                for f in gemm3_ops(*pend):
                    f()
            for sc in range(4):
                gemm4_piece(3, sc, 0)
                gemm4_piece(3, sc, 1)
    nc.finalize()
    return nc


# ---------------- legacy (dense/no-mask) builder, unchanged ----------------

def build_nc(mode="causal"):
    if mode == "causal":
        return build_nc2()
    raise NotImplementedError("v2 kernel supports the causal mask only")


_CACHE = {}


def _get_nc(mode):
    if mode not in _CACHE:
        _CACHE[mode] = build_nc2() if mode == "causal" else None
    return _CACHE[mode]


def _host_xt(x, bf):
    # xt[p, ss*8192 + dc*512 + si] = x[ss*512+si, dc*128+p]
    xr = np.asarray(x, np.float32).reshape(NSS, 512, NDC, 128)
    return np.ascontiguousarray(
        xr.transpose(3, 0, 2, 1).reshape(128, NDC * S).astype(bf))


def kernel(q, k, v, mask, Wq, bq, Wk, bk, Wv, bv, Wo, bo):
    q = np.asarray(q, np.float32)
    k = np.asarray(k, np.float32)
    v = np.asarray(v, np.float32)
    mask = np.asarray(mask)
    Wq = np.asarray(Wq, np.float32)
    Wk = np.asarray(Wk, np.float32)
    Wv = np.asarray(Wv, np.float32)
    Wo = np.asarray(Wo, np.float32)
    bq = np.asarray(bq, np.float32)
    bk = np.asarray(bk, np.float32)
    bv = np.asarray(bv, np.float32)
    bo = np.asarray(bo, np.float32)

    m = mask.astype(np.float64)
    assert np.array_equal(m, np.tril(np.ones((S, S)))), \
        "v2 kernel supports the causal mask"

    nc = _get_nc("causal")
    bf = ml_dtypes.bfloat16
    tri_np = np.triu(np.ones((128, 128))).astype(bf)
    id_np = np.eye(128).astype(bf)

    head_perm = [h for cc in range(4) for h in (cc, cc + 4)]
    col_perm = np.concatenate(
        [np.arange(h * HD, (h + 1) * HD) for h in head_perm])

    in_maps = []
    for core in range(NCORES):
        b, kb = core // KVSH, core % KVSH
        wq_sh = Wq[:, kb * CQ:(kb + 1) * CQ][:, col_perm]
        wo_sh = Wo[kb * CQ:(kb + 1) * CQ, :][col_perm, :]
        bq_sh = bq[kb * CQ:(kb + 1) * CQ][col_perm]
        wk_sh = Wk[:, kb * CK:(kb + 1) * CK]
        wv_sh = Wv[:, kb * CK:(kb + 1) * CK]
        # cc-major: wq_arr[p, cc*2048 + dc*128 + j]
        wq_arr = wq_sh.reshape(NDC, 128, 4, 128).transpose(1, 2, 0, 3).reshape(
            128, NDC * CQ)
        wkv_arr = np.stack(
            [w.reshape(NDC, 128, CK).transpose(1, 0, 2).reshape(128, NDC * CK)
             for w in (wk_sh, wv_sh)], axis=1).reshape(128, 2 * NDC * CK)
        wo_arr = wo_sh.reshape(4, 128, DIM).transpose(1, 0, 2).reshape(
            128, 4 * DIM)
        im = {
            "qt": _host_xt(q[b], bf),
            "kt": _host_xt(k[b], bf),
            "vt": _host_xt(v[b], bf),
            "wq": np.ascontiguousarray(wq_arr.astype(bf)),
            "wkv": np.ascontiguousarray(wkv_arr.astype(bf)),
            "wo": np.ascontiguousarray(wo_arr.astype(bf)),
            "bq": np.ascontiguousarray(bq_sh),
            "bk": np.ascontiguousarray(bk[kb * CK:(kb + 1) * CK]),
            "bv": np.ascontiguousarray(bv[kb * CK:(kb + 1) * CK]),
            "tri": tri_np,
            "ident": id_np,
        }
        in_maps.append(im)

    res = run_bass_kernel_spmd(nc, in_maps, core_ids=list(range(NCORES)))
    outs = [r["out"] for r in res.results]
    full = np.empty((B, S, DIM), np.float32)
    for b in range(B):
        acc = outs[b * KVSH].astype(np.float32)
        for kb in range(1, KVSH):
            acc = acc + outs[b * KVSH + kb].astype(np.float32)
        full[b] = acc + bo[None, :]
    return full


# revision 48
# speedup vs baseline: 1.0570x; 1.0422x over previous
"""Grouped-query attention (GQA) Trainium2 Bass kernel, v2.

Problem: B=2, S=2048, DIM=2048, HQ=32, HKV=8, HEAD_DIM=64, causal mask.
Sharding: 8 cores = 2 (batch) x 4 (kv-head groups). Core c handles batch
c//4 and kv-block c%4 (2 kv heads, 8 q heads). Wq/Wk/Wv sharded
column-wise, Wo row-wise; each core writes a partial [S, DIM] bf16
output; host sums the 4 partials per batch and adds bo.

v2 changes vs the previous kernel (all bf16; fp8 fails the 2e-2 gate):
  - Host pre-transposes q/k/v (x^T tiles streamed as plain wide DMAs;
    no XBAR dma transpose), DMA order tuned so the GEMM1 chains and the
    first heads' scores are never starved at startup.
  - exp emitted over [128, 1024] PSUM groups: full j-blocks paired, and
    the diagonal tail packed as [512|384|128] + [256] (one exp per
    group; the scores->exp->PSUM-recycle chain paces the pipeline, so
    fewer, wider ACT ops matter).
  - GEMM3 restructured: attn accumulated in natural [i, c] layout with
    exp blocks as the stationary operand and v(+ones) moving -> 65-row
    matmuls at full PE efficiency (~half the PE cycles of the j-layout),
    denominator lands as column 64 per i-partition. Runs two head-slots
    behind the score stream for ACT slack.
  - Normalization fused into the PSUM->SBUF copy: DVE fp32 reciprocal
    of the denominator column + per-partition tensor_scalar multiply.
    The PE broadcast-matmul normalize of v1 is gone.
  - attn^T for GEMM4 via PE transposes of the normalized [128, 64]
    chunks into one [64, 512] PSUM tile, one DVE copy per head back to
    SBUF. GEMM1 bias-adds on DVE; tri masking on GPSIMD (GPSIMD cannot
    touch PSUM); ACT does exp only.
PSUM: 4 banks score-groups (2x[128,1024]) + 2 banks gemm3 accumulators
+ 2 banks shared (gemm1 / vtrans / transposes / gemm4).
"""

import numpy as np
import ml_dtypes

import concourse.bass as bass
import concourse.mybir as mybir
from concourse import bacc
from concourse.tile import TileContext
from concourse.bass_utils import run_bass_kernel_spmd

F32 = mybir.dt.float32
BF16 = mybir.dt.bfloat16
AF = mybir.ActivationFunctionType
ALU = mybir.AluOpType

B, S, DIM = 2, 2048, 2048
HQ, HKV, HD = 32, 8, 64
GROUP = HQ // HKV              # 4
NCORES = 8
KVSH = 4                       # kv-blocks (shards) per batch
CQ = (HQ // KVSH) * HD         # 512 q-proj cols per core (8 heads)
CK = (HKV // KVSH) * HD        # 128 kv-proj cols per core (2 heads)
NDC = DIM // 128               # 16 contraction chunks
NSS = S // 512                 # 4 sequence chunks of 512


def build_nc2():
    """Causal-mode v2 builder."""
    nc = bacc.Bacc("TRN2", target_bir_lowering=False)

    # xt[p, ss*8192 + dc*512 + si] = x[ss*512+si, dc*128+p]
    qt = nc.dram_tensor("qt", [128, NDC * S], BF16, kind="ExternalInput")
    kt = nc.dram_tensor("kt", [128, NDC * S], BF16, kind="ExternalInput")
    vt = nc.dram_tensor("vt", [128, NDC * S], BF16, kind="ExternalInput")
    wq = nc.dram_tensor("wq", [128, NDC * CQ], BF16, kind="ExternalInput")
    wkv = nc.dram_tensor("wkv", [128, 2 * NDC * CK], BF16,
                         kind="ExternalInput")
    wo = nc.dram_tensor("wo", [128, 4 * DIM], BF16, kind="ExternalInput")
    bq = nc.dram_tensor("bq", [CQ], F32, kind="ExternalInput")
    bk = nc.dram_tensor("bk", [CK], F32, kind="ExternalInput")
    bv = nc.dram_tensor("bv", [CK], F32, kind="ExternalInput")
    tri = nc.dram_tensor("tri", [128, 128], BF16, kind="ExternalInput")
    ident = nc.dram_tensor("ident", [128, 128], BF16, kind="ExternalInput")
    out = nc.dram_tensor("out", [S, DIM], BF16, kind="ExternalOutput")

    XTS = {"q": qt, "k": kt, "v": vt}

    with TileContext(nc) as tc:
        with (
            tc.tile_pool(name="consts", bufs=1) as consts,
            tc.tile_pool(name="w", bufs=1) as wpool,
            tc.tile_pool(name="xt", bufs=1) as xt,
            tc.tile_pool(name="acts", bufs=1) as acts,
            tc.tile_pool(name="at2", bufs=2) as at2,
            tc.tile_pool(name="exp", bufs=21) as expp,
            tc.tile_pool(name="exps", bufs=4) as expsp,
            tc.tile_pool(name="nrm", bufs=6) as nrmp,
            tc.tile_pool(name="ob", bufs=5) as obp,
            tc.tile_pool(name="psc", bufs=2, space="PSUM") as psc,
            tc.tile_pool(name="psg", bufs=2, space="PSUM") as psg,
            tc.tile_pool(name="psx", bufs=2, space="PSUM") as psx,
        ):
            cn = {}

            def load_consts_early():
                cn["id"] = consts.tile([128, 128], BF16, tag="id",
                                       name="id_c")
                nc.sync.dma_start(out=cn["id"][:, :], in_=ident[:, :])
                cn["bk"] = consts.tile([128, 1], F32, tag="bk", name="bk_c")
                nc.sync.dma_start(
                    out=cn["bk"][:, :],
                    in_=bass.AP(tensor=bk[0:1].tensor, offset=0,
                                ap=[[1, 128], [128, 1]]))

            def load_consts():
                cn["tri"] = consts.tile([128, 128], BF16, tag="tri",
                                        name="tri_c")
                nc.sync.dma_start(out=cn["tri"][:, :], in_=tri[:, :])
                cn["bq"] = consts.tile([128, 4], F32, tag="bq", name="bq_c")
                nc.sync.dma_start(
                    out=cn["bq"][:, :],
                    in_=bass.AP(tensor=bq[0:1].tensor, offset=0,
                                ap=[[1, 128], [128, 4]]))
                cn["bv"] = consts.tile([128, 128], F32, tag="bv",
                                       name="bv_c")
                nc.sync.dma_start(
                    out=cn["bv"][:, :],
                    in_=bass.AP(tensor=bv[0:1].tensor, offset=0,
                                ap=[[0, 128], [1, 128]]))

            # ---- transposed input loads: plain DMA of host-packed x^T ----
            # Tiles cover [dc_lo, dc_hi) contraction chunks; finer tiles at
            # startup let the first GEMM1 chains begin sooner.
            HDC = NDC // 2
            XTB = {}

            def xtb_piece(ss, nm, dc_lo, dc_hi, tag):
                ndc = dc_hi - dc_lo
                t = xt.tile([128, ndc * 512], BF16, tag=tag,
                            name=f"x{tag}")
                c0 = ss * 8192 + dc_lo * 512
                nc.sync.dma_start(out=t[:, :],
                                  in_=XTS[nm][:, c0:c0 + ndc * 512])
                XTB.setdefault((ss, nm), []).append((dc_lo, dc_hi, t))

            def xtb_half(ss, nm, half):
                xtb_piece(ss, nm, half * HDC, (half + 1) * HDC,
                          f"x{nm}{half}")

            def xtb_load_t(ss, nm):
                xtb_half(ss, nm, 0)
                xtb_half(ss, nm, 1)

            def xtb_load(ss):
                for nm in "kvq":
                    xtb_load_t(ss, nm)

            def xslice(ss, nm, dc):
                for dc_lo, dc_hi, t in XTB[(ss, nm)]:
                    if dc_lo <= dc < dc_hi:
                        return t[:, (dc - dc_lo) * 512:(dc - dc_lo + 1) * 512]
                raise KeyError((ss, nm, dc))

            # ---- weights ----
            # prologue DMA order: k-chain first, then v (vx1 needed by the
            # first gemm3), then q per-cc chunks (cc-major wq layout).
            wk_bf = wpool.tile([128, NDC * CK], BF16, tag="wk", name="wk_bf")
            nc.sync.dma_start(out=wk_bf[:, :], in_=wkv[:, 0:2048])
            load_consts_early()
            for qt_ in range(4):
                xtb_piece(0, "k", qt_ * 4, (qt_ + 1) * 4, f"xk0q{qt_}")
            wv_bf = wpool.tile([128, NDC * CK], BF16, tag="wv", name="wv_bf")
            nc.sync.dma_start(out=wv_bf[:, :], in_=wkv[:, 2048:4096])
            xtb_load_t(0, "v")
            load_consts()
            wqc = []

            def load_wqc(cc):
                wq_c = wpool.tile([128, NDC * 128], BF16, tag=f"wq{cc}",
                                  name=f"wq{cc}")
                nc.sync.dma_start(out=wq_c[:, :],
                                  in_=wq[:, cc * 2048:(cc + 1) * 2048])
                wqc.append(wq_c)

            xtb_half(0, "q", 0)
            load_wqc(0)
            xtb_half(0, "q", 1)
            for cc in range(1, 4):
                load_wqc(cc)
            wo_bf = wpool.tile([128, 4 * DIM], BF16, tag="wo", name="wo_bf")
            nc.sync.dma_start(out=wo_bf[:, :], in_=wo[:, :])

            # ---- persistent activations ----
            qxT = [acts.tile([128, S], BF16, tag=f"qx{cc}", name=f"qx{cc}")
                   for cc in range(4)]
            kxT = acts.tile([128, S], BF16, tag="kx", name="kx")
            vxT = acts.tile([128, S], BF16, tag="vx", name="vx")
            vx1 = [acts.tile([128, 130], BF16, tag=f"vp{sc}", name=f"vp{sc}")
                   for sc in range(S // 128)]

            def attnT(ss, cc):
                # double-buffered across ss (gemm4 runs one block behind)
                return at2.tile([128, 512], BF16, tag=f"at{cc}",
                                name=f"at{ss}{cc}")

            attnTs = {}

            def gemm1_q_cc(ss, cc):
                s0 = ss * 512
                ps = psx.tile([128, 512], F32, tag="m")
                for dc in range(NDC):
                    nc.tensor.matmul(
                        ps[:, :],
                        wqc[cc][:, dc * 128:(dc + 1) * 128],
                        xslice(ss, "q", dc),
                        start=(dc == 0), stop=(dc == NDC - 1))
                nc.vector.tensor_scalar_add(qxT[cc][:, s0:s0 + 512],
                                            ps[:, :], cn["bq"][:, cc:cc + 1])

            def gemm1_k(ss):
                s0 = ss * 512
                ps = psx.tile([128, 512], F32, tag="m")
                for dc in range(NDC):
                    nc.tensor.matmul(
                        ps[:, :], wk_bf[:, dc * 128:(dc + 1) * 128],
                        xslice(ss, "k", dc),
                        start=(dc == 0), stop=(dc == NDC - 1))
                nc.vector.tensor_scalar_add(kxT[:, s0:s0 + 512], ps[:, :],
                                            cn["bk"][:, 0:1])

            def gemm1_v(ss):
                s0 = ss * 512
                ps = psx.tile([128, 512], F32, tag="m")
                for dc in range(NDC):
                    nc.tensor.matmul(
                        ps[:, :], wv_bf[:, dc * 128:(dc + 1) * 128],
                        xslice(ss, "v", dc),
                        start=(dc == 0), stop=(dc == NDC - 1))
                nc.vector.tensor_copy(vxT[:, s0:s0 + 512], ps[:, :])

            def vtrans(ss):
                s0 = ss * 512
                vtp = psx.tile([128, 512], BF16, tag="m")
                for sc in range(4):
                    nc.tensor.transpose(
                        vtp[:, sc * 128:(sc + 1) * 128],
                        vxT[:, s0 + sc * 128:s0 + (sc + 1) * 128],
                        cn["id"][:, :])
                for sc in range(4):
                    jb = ss * 4 + sc
                    vx = vx1[jb]
                    for h2 in range(2):
                        nc.vector.tensor_tensor(
                            vx[:, h2 * 65:h2 * 65 + 64],
                            vtp[:, sc * 128 + h2 * 64:sc * 128 + (h2 + 1) * 64],
                            cn["bv"][:, h2 * 64:(h2 + 1) * 64], ALU.add)
                    nc.vector.memset(vx[:, 64:65], 1.0)
                    nc.vector.memset(vx[:, 129:130], 1.0)

            # exinfo[(ss, h, jb)] = (sbuf exp tile, col0, off)
            exinfo = {}

            def scores_grp(ss, h, grp):
                """One PSUM tile holding the given (jb, col0) j-blocks
                packed back-to-back: matmuls + a single exp (+ tri for
                diagonal blocks)."""
                s0 = ss * 512
                th, po, kv = h % 4, (h // GROUP) * 64, h // GROUP
                tot = max(c0 + 512 - max(0, jb * 128 - s0)
                          for jb, c0 in grp)
                sp = psc.tile([128, 1024], F32, tag="sc")
                if tot > 512:
                    ex = expp.tile([128, 1024], BF16, tag="exp")
                else:
                    ex = expsp.tile([128, 512], BF16, tag="exps")
                for jb, c0 in grp:
                    j0 = jb * 128
                    off = max(0, j0 - s0)
                    N = 512 - off
                    nc.tensor.matmul(
                        sp[:, c0:c0 + N],
                        kxT[kv * 64:(kv + 1) * 64, j0:j0 + 128],
                        qxT[th][po:po + 64, s0 + off:s0 + 512],
                        start=True, stop=True)
                    exinfo[(ss, h, jb)] = (ex, c0, off)
                nc.scalar.activation(ex[:, :tot], sp[:, :tot], AF.Exp,
                                     scale=0.125)
                for jb, c0 in grp:
                    if jb >= 4 * ss:
                        nc.gpsimd.tensor_tensor(
                            ex[:, c0:c0 + 128], ex[:, c0:c0 + 128],
                            cn["tri"][:, :], ALU.mult)

            def scores_plan(ss):
                """Full blocks paired; the diagonal tail packed as
                [512|384|128] (exactly one [128,1024] tile) + [256]."""
                d = 4 * ss
                grps = [[(jb, 0), (jb + 1, 512)] for jb in range(0, d, 2)]
                grps.append([(d, 0), (d + 1, 512), (d + 3, 896)])
                grps.append([(d + 2, 0)])
                return grps

            def gemm3_chunk(ss, h, sc):
                """attn chunk [128 i, 65] for i-block ib = 4ss+sc; returns
                psum tile."""
                kv = h // GROUP
                ib = 4 * ss + sc
                at = psg.tile([128, 512], F32, tag="g3")
                for jb in range(ib + 1):
                    ex, c0, off = exinfo[(ss, h, jb)]
                    nc.tensor.matmul(
                        at[:, 0:65],
                        ex[:, c0 + sc * 128 - off:c0 + sc * 128 - off + 128],
                        vx1[jb][:, kv * 65:kv * 65 + 65],
                        start=(jb == 0), stop=(jb == ib))
                return at

            def gemm3_norm(ss, h, sc, at):
                """fp32 reciprocal + fused normalize into SBUF copy."""
                rcp = nrmp.tile([128, 1], F32, tag="rcp")
                nc.vector.reciprocal(rcp[:, :], at[:, 64:65])
                an = nrmp.tile([128, 64], BF16, tag="an")
                nc.vector.tensor_scalar_mul(an[:, :], at[:, 0:64], rcp[:, :])
                return an

            def gemm3_ops(ss, h):
                """5 closures: chunk+norm x4 with transposes delayed so
                the DVE norm is long done, then one [64, 512] copy into
                attnT[ss]."""
                th, po = h % 4, (h // GROUP) * 64
                st = {}

                def chunk(sc):
                    def f():
                        at = gemm3_chunk(ss, h, sc)
                        st[sc] = gemm3_norm(ss, h, sc, at)
                        if sc == 2:
                            st["tr"] = psx.tile([64, 512], BF16, tag="m",
                                                name="tr")
                            for lo in (0, 1):
                                nc.tensor.transpose(
                                    st["tr"][0:64, lo * 128:(lo + 1) * 128],
                                    st[lo][:, :], cn["id"][:, :])
                        elif sc == 3:
                            nc.tensor.transpose(
                                st["tr"][0:64, 256:384],
                                st[2][:, :], cn["id"][:, :])
                    return f

                def fin():
                    nc.tensor.transpose(st["tr"][0:64, 384:512],
                                        st[3][:, :], cn["id"][:, :])
                    nc.vector.tensor_copy(
                        attnTs[(ss, th)][po:po + 64, :], st["tr"][0:64, :])

                return [chunk(0), chunk(1), chunk(2), chunk(3), fin]

            def gemm4_piece(ss, sc, eh):
                s0 = ss * 512
                i0 = s0 + sc * 128
                ob = obp.tile([128, 1024], BF16, tag="ob")
                for e2 in range(2):
                    ec = eh * 2 + e2
                    g4 = psx.tile([128, 512], F32, tag="m")
                    for cc2 in range(4):
                        nc.tensor.matmul(
                            g4[:, :],
                            attnTs[(ss, cc2)][:, sc * 128:(sc + 1) * 128],
                            wo_bf[:, cc2 * 2048 + ec * 512:
                                  cc2 * 2048 + (ec + 1) * 512],
                            start=(cc2 == 0), stop=(cc2 == 3))
                    nc.vector.tensor_copy(
                        ob[:, e2 * 512:(e2 + 1) * 512], g4[:, :])
                nc.sync.dma_start(
                    out=out[i0:i0 + 128, eh * 1024:(eh + 1) * 1024],
                    in_=ob[:, :])

            def block(ss, extra, carry_in):
                """scores(h) + gemm3(h-1) pipeline; carry_in = (pss, 7) of
                the previous block's last head, processed at h==0."""
                for cc in range(4):
                    attnTs[(ss, cc)] = attnT(ss, cc)
                if ss == 0:
                    gemm1_k(ss)
                    gemm1_v(ss)
                    vtrans(ss)
                    gemm1_q_cc(ss, 0)
                else:
                    gemm1_k(ss)
                    gemm1_v(ss)
                grps = scores_plan(ss)
                pending = list(carry_in or [])
                for h in range(8):
                    if h == 0 and ss > 0:
                        gemm1_q_cc(ss, 0)
                        vtrans(ss)
                        gemm1_q_cc(ss, 1)
                    elif h == 0:
                        gemm1_q_cc(ss, 1)
                    elif h == 1:
                        gemm1_q_cc(ss, 2)
                    elif h == 2:
                        gemm1_q_cc(ss, 3)
                    # interleave: scores groups of head h with gemm3 of the
                    # head TWO slots back (extra ACT slack) and this slot's
                    # extra ops (gemm4 etc.), fills front-loaded.
                    ng = len(grps)
                    g3ops = []
                    if len(pending) >= 2:
                        g3ops = gemm3_ops(*pending.pop(0))
                    fill = list(g3ops) + list(extra.get(h, ()))
                    n_emit, n_tot = 0, len(fill)
                    for gi, grp in enumerate(grps):
                        want = n_tot * gi // ng
                        while n_emit < want:
                            fill[n_emit]()
                            n_emit += 1
                        scores_grp(ss, h, grp)
                    while n_emit < n_tot:
                        fill[n_emit]()
                        n_emit += 1
                    pending.append((ss, h))
                return pending

            # ---- schedule ----
            xtb_load(1)
            carry = block(0, {}, None)

            ext1 = {h: [lambda h=h: gemm4_piece(0, (h - 1) // 2,
                                                (h - 1) % 2)]
                    for h in range(1, 8)}
            ext1[3].append(lambda: xtb_load_t(2, "k"))
            ext1[4].append(lambda: xtb_load_t(2, "v"))
            ext1[5].append(lambda: xtb_load_t(2, "q"))
            carry = block(1, ext1, carry)
            gemm4_piece(0, 3, 1)

            ext2 = {h: [lambda h=h: gemm4_piece(1, (h - 1) // 2,
                                                (h - 1) % 2)]
                    for h in range(1, 8)}
            ext2[3].append(lambda: xtb_load_t(3, "k"))
            ext2[4].append(lambda: xtb_load_t(3, "v"))
            ext2[5].append(lambda: xtb_load_t(3, "q"))
            carry = block(2, ext2, carry)
            gemm4_piece(1, 3, 1)

            ext3 = {h: [lambda h=h: gemm4_piece(2, (h - 1) // 2,
                                                (h - 1) % 2)]
                    for h in range(1, 8)}
            carry = block(3, ext3, carry)
            gemm4_piece(2, 3, 1)
            # drain: the last two heads' gemm3, then block 3's gemm4
            for pend in carry:
                for f in gemm3_ops(*pend):
                    f()
            for sc in range(4):
                gemm4_piece(3, sc, 0)
                gemm4_piece(3, sc, 1)
    nc.finalize()
    return nc


# ---------------- legacy (dense/no-mask) builder, unchanged ----------------

def build_nc(mode="causal"):
    if mode == "causal":
        return build_nc2()
    raise NotImplementedError("v2 kernel supports the causal mask only")


_CACHE = {}


def _get_nc(mode):
    if mode not in _CACHE:
        _CACHE[mode] = build_nc2() if mode == "causal" else None
    return _CACHE[mode]


def _host_xt(x, bf):
    # xt[p, ss*8192 + dc*512 + si] = x[ss*512+si, dc*128+p]
    xr = np.asarray(x, np.float32).reshape(NSS, 512, NDC, 128)
    return np.ascontiguousarray(
        xr.transpose(3, 0, 2, 1).reshape(128, NDC * S).astype(bf))


def kernel(q, k, v, mask, Wq, bq, Wk, bk, Wv, bv, Wo, bo):
    q = np.asarray(q, np.float32)
    k = np.asarray(k, np.float32)
    v = np.asarray(v, np.float32)
    mask = np.asarray(mask)
    Wq = np.asarray(Wq, np.float32)
    Wk = np.asarray(Wk, np.float32)
    Wv = np.asarray(Wv, np.float32)
    Wo = np.asarray(Wo, np.float32)
    bq = np.asarray(bq, np.float32)
    bk = np.asarray(bk, np.float32)
    bv = np.asarray(bv, np.float32)
    bo = np.asarray(bo, np.float32)

    m = mask.astype(np.float64)
    assert np.array_equal(m, np.tril(np.ones((S, S)))), \
        "v2 kernel supports the causal mask"

    nc = _get_nc("causal")
    bf = ml_dtypes.bfloat16
    tri_np = np.triu(np.ones((128, 128))).astype(bf)
    id_np = np.eye(128).astype(bf)

    head_perm = [h for cc in range(4) for h in (cc, cc + 4)]
    col_perm = np.concatenate(
        [np.arange(h * HD, (h + 1) * HD) for h in head_perm])

    in_maps = []
    for core in range(NCORES):
        b, kb = core // KVSH, core % KVSH
        wq_sh = Wq[:, kb * CQ:(kb + 1) * CQ][:, col_perm]
        wo_sh = Wo[kb * CQ:(kb + 1) * CQ, :][col_perm, :]
        bq_sh = bq[kb * CQ:(kb + 1) * CQ][col_perm]
        wk_sh = Wk[:, kb * CK:(kb + 1) * CK]
        wv_sh = Wv[:, kb * CK:(kb + 1) * CK]
        # cc-major: wq_arr[p, cc*2048 + dc*128 + j]
        wq_arr = wq_sh.reshape(NDC, 128, 4, 128).transpose(1, 2, 0, 3).reshape(
            128, NDC * CQ)
        wkv_arr = np.stack(
            [w.reshape(NDC, 128, CK).transpose(1, 0, 2).reshape(128, NDC * CK)
             for w in (wk_sh, wv_sh)], axis=1).reshape(128, 2 * NDC * CK)
        wo_arr = wo_sh.reshape(4, 128, DIM).transpose(1, 0, 2).reshape(
            128, 4 * DIM)
        im = {
            "qt": _host_xt(q[b], bf),
            "kt": _host_xt(k[b], bf),
            "vt": _host_xt(v[b], bf),
            "wq": np.ascontiguousarray(wq_arr.astype(bf)),
            "wkv": np.ascontiguousarray(wkv_arr.astype(bf)),
            "wo": np.ascontiguousarray(wo_arr.astype(bf)),
            "bq": np.ascontiguousarray(bq_sh),
            "bk": np.ascontiguousarray(bk[kb * CK:(kb + 1) * CK]),
            "bv": np.ascontiguousarray(bv[kb * CK:(kb + 1) * CK]),
            "tri": tri_np,
            "ident": id_np,
        }
        in_maps.append(im)

    res = run_bass_kernel_spmd(nc, in_maps, core_ids=list(range(NCORES)))
    outs = [r["out"] for r in res.results]
    full = np.empty((B, S, DIM), np.float32)
    for b in range(B):
        acc = outs[b * KVSH].astype(np.float32)
        for kb in range(1, KVSH):
            acc = acc + outs[b * KVSH + kb].astype(np.float32)
        full[b] = acc + bo[None, :]
    return full


# revision 52
# speedup vs baseline: 1.0734x; 1.0155x over previous
"""Grouped-query attention (GQA) Trainium2 Bass kernel, v2.

Problem: B=2, S=2048, DIM=2048, HQ=32, HKV=8, HEAD_DIM=64, causal mask.
Sharding: 8 cores = 2 (batch) x 4 (kv-head groups). Core c handles batch
c//4 and kv-block c%4 (2 kv heads, 8 q heads). Wq/Wk/Wv sharded
column-wise, Wo row-wise; each core writes a partial [S, DIM] bf16
output; host sums the 4 partials per batch and adds bo.

v2 changes vs the previous kernel (all bf16; fp8 fails the 2e-2 gate):
  - Host pre-transposes q/k/v (x^T tiles streamed as plain wide DMAs;
    no XBAR dma transpose), DMA order tuned so the GEMM1 chains and the
    first heads' scores are never starved at startup.
  - exp emitted over [128, 1024] PSUM groups: full j-blocks paired, and
    the diagonal tail packed as [512|384|128] + [256] (one exp per
    group; the scores->exp->PSUM-recycle chain paces the pipeline, so
    fewer, wider ACT ops matter).
  - GEMM3 restructured: attn accumulated in natural [i, c] layout with
    exp blocks as the stationary operand and v(+ones) moving -> 65-row
    matmuls at full PE efficiency (~half the PE cycles of the j-layout),
    denominator lands as column 64 per i-partition. Runs two head-slots
    behind the score stream for ACT slack.
  - Normalization fused into the PSUM->SBUF copy: DVE fp32 reciprocal
    of the denominator column + per-partition tensor_scalar multiply.
    The PE broadcast-matmul normalize of v1 is gone.
  - attn^T for GEMM4 via PE transposes of the normalized [128, 64]
    chunks into one [64, 512] PSUM tile, one DVE copy per head back to
    SBUF. GEMM1 bias-adds on DVE; tri masking on GPSIMD (GPSIMD cannot
    touch PSUM); ACT does exp only.
PSUM: 4 banks score-groups (2x[128,1024]) + 2 banks gemm3 accumulators
+ 2 banks shared (gemm1 / vtrans / transposes / gemm4).
"""

import numpy as np
import ml_dtypes

import concourse.bass as bass
import concourse.mybir as mybir
from concourse import bacc
from concourse.tile import TileContext
from concourse.bass_utils import run_bass_kernel_spmd

F32 = mybir.dt.float32
BF16 = mybir.dt.bfloat16
AF = mybir.ActivationFunctionType
ALU = mybir.AluOpType

B, S, DIM = 2, 2048, 2048
HQ, HKV, HD = 32, 8, 64
GROUP = HQ // HKV              # 4
NCORES = 8
KVSH = 4                       # kv-blocks (shards) per batch
CQ = (HQ // KVSH) * HD         # 512 q-proj cols per core (8 heads)
CK = (HKV // KVSH) * HD        # 128 kv-proj cols per core (2 heads)
NDC = DIM // 128               # 16 contraction chunks
NSS = S // 512                 # 4 sequence chunks of 512


def build_nc2():
    """Causal-mode v2 builder."""
    nc = bacc.Bacc("TRN2", target_bir_lowering=False)

    # xt[p, ss*8192 + dc*512 + si] = x[ss*512+si, dc*128+p]
    qt = nc.dram_tensor("qt", [128, NDC * S], BF16, kind="ExternalInput")
    kt = nc.dram_tensor("kt", [128, NDC * S], BF16, kind="ExternalInput")
    vt = nc.dram_tensor("vt", [128, NDC * S], BF16, kind="ExternalInput")
    wq = nc.dram_tensor("wq", [128, NDC * CQ], BF16, kind="ExternalInput")
    wkv = nc.dram_tensor("wkv", [128, 2 * NDC * CK], BF16,
                         kind="ExternalInput")
    wo = nc.dram_tensor("wo", [128, 4 * DIM], BF16, kind="ExternalInput")
    bq = nc.dram_tensor("bq", [CQ], F32, kind="ExternalInput")
    bk = nc.dram_tensor("bk", [CK], F32, kind="ExternalInput")
    bv = nc.dram_tensor("bv", [CK], F32, kind="ExternalInput")
    tri = nc.dram_tensor("tri", [128, 128], BF16, kind="ExternalInput")
    ident = nc.dram_tensor("ident", [128, 128], BF16, kind="ExternalInput")
    out = nc.dram_tensor("out", [S, DIM], BF16, kind="ExternalOutput")

    XTS = {"q": qt, "k": kt, "v": vt}

    with TileContext(nc) as tc:
        with (
            tc.tile_pool(name="consts", bufs=1) as consts,
            tc.tile_pool(name="w", bufs=1) as wpool,
            tc.tile_pool(name="xt", bufs=1) as xt,
            tc.tile_pool(name="acts", bufs=1) as acts,
            tc.tile_pool(name="at2", bufs=3) as at2,
            tc.tile_pool(name="exp", bufs=21) as expp,
            tc.tile_pool(name="exps", bufs=4) as expsp,
            tc.tile_pool(name="nrm", bufs=6) as nrmp,
            tc.tile_pool(name="ob", bufs=5) as obp,
            tc.tile_pool(name="psc", bufs=2, space="PSUM") as psc,
            tc.tile_pool(name="psg", bufs=2, space="PSUM") as psg,
            tc.tile_pool(name="psx", bufs=2, space="PSUM") as psx,
        ):
            cn = {}

            def load_consts_early():
                cn["id"] = consts.tile([128, 128], BF16, tag="id",
                                       name="id_c")
                nc.sync.dma_start(out=cn["id"][:, :], in_=ident[:, :])
                cn["bk"] = consts.tile([128, 1], F32, tag="bk", name="bk_c")
                nc.sync.dma_start(
                    out=cn["bk"][:, :],
                    in_=bass.AP(tensor=bk[0:1].tensor, offset=0,
                                ap=[[1, 128], [128, 1]]))

            def load_consts():
                cn["tri"] = consts.tile([128, 128], BF16, tag="tri",
                                        name="tri_c")
                nc.sync.dma_start(out=cn["tri"][:, :], in_=tri[:, :])
                cn["bq"] = consts.tile([128, 4], F32, tag="bq", name="bq_c")
                nc.sync.dma_start(
                    out=cn["bq"][:, :],
                    in_=bass.AP(tensor=bq[0:1].tensor, offset=0,
                                ap=[[1, 128], [128, 4]]))
                cn["bv"] = consts.tile([128, 128], F32, tag="bv",
                                       name="bv_c")
                nc.sync.dma_start(
                    out=cn["bv"][:, :],
                    in_=bass.AP(tensor=bv[0:1].tensor, offset=0,
                                ap=[[0, 128], [1, 128]]))

            # ---- transposed input loads: plain DMA of host-packed x^T ----
            # Tiles cover [dc_lo, dc_hi) contraction chunks; finer tiles at
            # startup let the first GEMM1 chains begin sooner.
            HDC = NDC // 2
            XTB = {}

            def xtb_piece(ss, nm, dc_lo, dc_hi, tag):
                ndc = dc_hi - dc_lo
                t = xt.tile([128, ndc * 512], BF16, tag=tag,
                            name=f"x{tag}")
                c0 = ss * 8192 + dc_lo * 512
                nc.sync.dma_start(out=t[:, :],
                                  in_=XTS[nm][:, c0:c0 + ndc * 512])
                XTB.setdefault((ss, nm), []).append((dc_lo, dc_hi, t))

            def xtb_half(ss, nm, half):
                xtb_piece(ss, nm, half * HDC, (half + 1) * HDC,
                          f"x{nm}{half}")

            def xtb_load_t(ss, nm):
                xtb_half(ss, nm, 0)
                xtb_half(ss, nm, 1)

            def xtb_load(ss):
                for nm in "kvq":
                    xtb_load_t(ss, nm)

            def xslice(ss, nm, dc):
                for dc_lo, dc_hi, t in XTB[(ss, nm)]:
                    if dc_lo <= dc < dc_hi:
                        return t[:, (dc - dc_lo) * 512:(dc - dc_lo + 1) * 512]
                raise KeyError((ss, nm, dc))

            # ---- weights ----
            # prologue DMA order: k-chain first, then v (vx1 needed by the
            # first gemm3), then q per-cc chunks (cc-major wq layout).
            wk_bf = wpool.tile([128, NDC * CK], BF16, tag="wk", name="wk_bf")
            nc.sync.dma_start(out=wk_bf[:, :], in_=wkv[:, 0:2048])
            load_consts_early()
            for qt_ in range(4):
                xtb_piece(0, "k", qt_ * 4, (qt_ + 1) * 4, f"xk0q{qt_}")
            wv_bf = wpool.tile([128, NDC * CK], BF16, tag="wv", name="wv_bf")
            nc.sync.dma_start(out=wv_bf[:, :], in_=wkv[:, 2048:4096])
            xtb_load_t(0, "v")
            load_consts()
            wqc = []

            def load_wqc(cc):
                wq_c = wpool.tile([128, NDC * 128], BF16, tag=f"wq{cc}",
                                  name=f"wq{cc}")
                nc.sync.dma_start(out=wq_c[:, :],
                                  in_=wq[:, cc * 2048:(cc + 1) * 2048])
                wqc.append(wq_c)

            xtb_half(0, "q", 0)
            load_wqc(0)
            xtb_half(0, "q", 1)
            for cc in range(1, 4):
                load_wqc(cc)
            wo_bf = wpool.tile([128, 4 * DIM], BF16, tag="wo", name="wo_bf")
            nc.sync.dma_start(out=wo_bf[:, :], in_=wo[:, :])

            # ---- persistent activations ----
            qxT = [acts.tile([128, S], BF16, tag=f"qx{cc}", name=f"qx{cc}")
                   for cc in range(4)]
            kxT = acts.tile([128, S], BF16, tag="kx", name="kx")
            vxT = acts.tile([128, S], BF16, tag="vx", name="vx")
            vx1 = [acts.tile([128, 130], BF16, tag=f"vp{sc}", name=f"vp{sc}")
                   for sc in range(S // 128)]

            def attnT(ss, cc):
                # double-buffered across ss (gemm4 runs one block behind)
                return at2.tile([128, 512], BF16, tag=f"at{cc}",
                                name=f"at{ss}{cc}")

            attnTs = {}

            def gemm1_q_cc(ss, cc):
                s0 = ss * 512
                ps = psx.tile([128, 512], F32, tag="m")
                for dc in range(NDC):
                    nc.tensor.matmul(
                        ps[:, :],
                        wqc[cc][:, dc * 128:(dc + 1) * 128],
                        xslice(ss, "q", dc),
                        start=(dc == 0), stop=(dc == NDC - 1))
                nc.vector.tensor_scalar_add(qxT[cc][:, s0:s0 + 512],
                                            ps[:, :], cn["bq"][:, cc:cc + 1])

            def gemm1_k(ss):
                s0 = ss * 512
                ps = psx.tile([128, 512], F32, tag="m")
                for dc in range(NDC):
                    nc.tensor.matmul(
                        ps[:, :], wk_bf[:, dc * 128:(dc + 1) * 128],
                        xslice(ss, "k", dc),
                        start=(dc == 0), stop=(dc == NDC - 1))
                nc.vector.tensor_scalar_add(kxT[:, s0:s0 + 512], ps[:, :],
                                            cn["bk"][:, 0:1])

            def gemm1_v(ss):
                s0 = ss * 512
                ps = psx.tile([128, 512], F32, tag="m")
                for dc in range(NDC):
                    nc.tensor.matmul(
                        ps[:, :], wv_bf[:, dc * 128:(dc + 1) * 128],
                        xslice(ss, "v", dc),
                        start=(dc == 0), stop=(dc == NDC - 1))
                nc.vector.tensor_copy(vxT[:, s0:s0 + 512], ps[:, :])

            def vtrans(ss):
                s0 = ss * 512
                vtp = psx.tile([128, 512], BF16, tag="m")
                for sc in range(4):
                    nc.tensor.transpose(
                        vtp[:, sc * 128:(sc + 1) * 128],
                        vxT[:, s0 + sc * 128:s0 + (sc + 1) * 128],
                        cn["id"][:, :])
                for sc in range(4):
                    jb = ss * 4 + sc
                    vx = vx1[jb]
                    for h2 in range(2):
                        nc.vector.tensor_tensor(
                            vx[:, h2 * 65:h2 * 65 + 64],
                            vtp[:, sc * 128 + h2 * 64:sc * 128 + (h2 + 1) * 64],
                            cn["bv"][:, h2 * 64:(h2 + 1) * 64], ALU.add)
                    nc.vector.memset(vx[:, 64:65], 1.0)
                    nc.vector.memset(vx[:, 129:130], 1.0)

            # exinfo[(ss, h, jb)] = (sbuf exp tile, col0, off)
            exinfo = {}

            def scores_grp(ss, h, grp):
                """One PSUM tile holding the given (jb, col0) j-blocks
                packed back-to-back: matmuls + a single exp (+ tri for
                diagonal blocks)."""
                s0 = ss * 512
                th, po, kv = h % 4, (h // GROUP) * 64, h // GROUP
                tot = max(c0 + 512 - max(0, jb * 128 - s0)
                          for jb, c0 in grp)
                sp = psc.tile([128, 1024], F32, tag="sc")
                if tot > 512:
                    ex = expp.tile([128, 1024], BF16, tag="exp")
                else:
                    ex = expsp.tile([128, 512], BF16, tag="exps")
                for jb, c0 in grp:
                    j0 = jb * 128
                    off = max(0, j0 - s0)
                    N = 512 - off
                    nc.tensor.matmul(
                        sp[:, c0:c0 + N],
                        kxT[kv * 64:(kv + 1) * 64, j0:j0 + 128],
                        qxT[th][po:po + 64, s0 + off:s0 + 512],
                        start=True, stop=True)
                    exinfo[(ss, h, jb)] = (ex, c0, off)
                nc.scalar.activation(ex[:, :tot], sp[:, :tot], AF.Exp,
                                     scale=0.125)
                for jb, c0 in grp:
                    if jb >= 4 * ss:
                        nc.gpsimd.tensor_tensor(
                            ex[:, c0:c0 + 128], ex[:, c0:c0 + 128],
                            cn["tri"][:, :], ALU.mult)

            def scores_plan(ss):
                """Full blocks paired; the diagonal tail packed as
                [512|384|128] (exactly one [128,1024] tile) + [256]."""
                d = 4 * ss
                grps = [[(jb, 0), (jb + 1, 512)] for jb in range(0, d, 2)]
                grps.append([(d, 0), (d + 1, 512), (d + 3, 896)])
                grps.append([(d + 2, 0)])
                return grps

            def gemm3_chunk(ss, h, sc):
                """attn chunk [128 i, 65] for i-block ib = 4ss+sc; returns
                psum tile."""
                kv = h // GROUP
                ib = 4 * ss + sc
                at = psg.tile([128, 512], F32, tag="g3")
                for jb in range(ib + 1):
                    ex, c0, off = exinfo[(ss, h, jb)]
                    nc.tensor.matmul(
                        at[:, 0:65],
                        ex[:, c0 + sc * 128 - off:c0 + sc * 128 - off + 128],
                        vx1[jb][:, kv * 65:kv * 65 + 65],
                        start=(jb == 0), stop=(jb == ib))
                return at

            def gemm3_norm(ss, h, sc, at):
                """fp32 reciprocal + fused normalize into SBUF copy."""
                rcp = nrmp.tile([128, 1], F32, tag="rcp")
                nc.vector.reciprocal(rcp[:, :], at[:, 64:65])
                an = nrmp.tile([128, 64], BF16, tag="an")
                nc.vector.tensor_scalar_mul(an[:, :], at[:, 0:64], rcp[:, :])
                return an

            def gemm3_ops(ss, h):
                """5 closures: chunk+norm x4 with transposes delayed so
                the DVE norm is long done, then one [64, 512] copy into
                attnT[ss]."""
                th, po = h % 4, (h // GROUP) * 64
                st = {}

                def chunk(sc):
                    def f():
                        at = gemm3_chunk(ss, h, sc)
                        st[sc] = gemm3_norm(ss, h, sc, at)
                        if sc == 2:
                            st["tr"] = psx.tile([64, 512], BF16, tag="m",
                                                name="tr")
                            for lo in (0, 1):
                                nc.tensor.transpose(
                                    st["tr"][0:64, lo * 128:(lo + 1) * 128],
                                    st[lo][:, :], cn["id"][:, :])
                        elif sc == 3:
                            nc.tensor.transpose(
                                st["tr"][0:64, 256:384],
                                st[2][:, :], cn["id"][:, :])
                    return f

                def fin():
                    nc.tensor.transpose(st["tr"][0:64, 384:512],
                                        st[3][:, :], cn["id"][:, :])
                    nc.vector.tensor_copy(
                        attnTs[(ss, th)][po:po + 64, :], st["tr"][0:64, :])

                return [chunk(0), chunk(1), chunk(2), chunk(3), fin]

            def gemm4_piece(ss, sc, eh):
                s0 = ss * 512
                i0 = s0 + sc * 128
                ob = obp.tile([128, 1024], BF16, tag="ob")
                for e2 in range(2):
                    ec = eh * 2 + e2
                    g4 = psx.tile([128, 512], F32, tag="m")
                    for cc2 in range(4):
                        nc.tensor.matmul(
                            g4[:, :],
                            attnTs[(ss, cc2)][:, sc * 128:(sc + 1) * 128],
                            wo_bf[:, cc2 * 2048 + ec * 512:
                                  cc2 * 2048 + (ec + 1) * 512],
                            start=(cc2 == 0), stop=(cc2 == 3))
                    nc.vector.tensor_copy(
                        ob[:, e2 * 512:(e2 + 1) * 512], g4[:, :])
                nc.sync.dma_start(
                    out=out[i0:i0 + 128, eh * 1024:(eh + 1) * 1024],
                    in_=ob[:, :])

            def block(ss, extra, carry_in):
                """scores(h) + gemm3(h-1) pipeline; carry_in = (pss, 7) of
                the previous block's last head, processed at h==0."""
                for cc in range(4):
                    attnTs[(ss, cc)] = attnT(ss, cc)
                if ss == 0:
                    gemm1_k(ss)
                    gemm1_v(ss)
                    vtrans(ss)
                    gemm1_q_cc(ss, 0)
                else:
                    gemm1_k(ss)
                    gemm1_v(ss)
                grps = scores_plan(ss)
                pending = list(carry_in or [])
                for h in range(8):
                    # q0 must precede this slot's scores; the rest of the
                    # projection work rides the fill list so the carried
                    # gemm3's DVE norm ops are served first (psg recycle).
                    tail_ops = []
                    if h == 0 and ss > 0:
                        gemm1_q_cc(ss, 0)
                        tail_ops = [lambda: vtrans(ss),
                                    lambda: gemm1_q_cc(ss, 1)]
                    elif h == 0:
                        tail_ops = [lambda: gemm1_q_cc(ss, 1)]
                    elif h == 1:
                        tail_ops = [lambda: gemm1_q_cc(ss, 2)]
                    elif h == 2:
                        tail_ops = [lambda: gemm1_q_cc(ss, 3)]
                    # interleave: scores groups of head h with gemm3 of the
                    # head TWO slots back (extra ACT slack) and this slot's
                    # extra ops (gemm4 etc.), fills front-loaded.
                    ng = len(grps)
                    g3ops = []
                    if len(pending) >= 2:
                        g3ops = gemm3_ops(*pending.pop(0))
                    fill = list(g3ops) + tail_ops + list(extra.get(h, ()))
                    n_emit, n_tot = 0, len(fill)
                    for gi, grp in enumerate(grps):
                        scores_grp(ss, h, grp)
                        want = min(n_tot, -(-n_tot * (gi + 1) // ng))
                        while n_emit < want:
                            fill[n_emit]()
                            n_emit += 1
                    while n_emit < n_tot:
                        fill[n_emit]()
                        n_emit += 1
                    pending.append((ss, h))
                return pending

            # ---- schedule ----
            xtb_load(1)
            carry = block(0, {}, None)

            ext1 = {h: [lambda h=h: gemm4_piece(0, (h - 1) // 2,
                                                (h - 1) % 2)]
                    for h in range(1, 8)}
            ext1[3].append(lambda: xtb_load_t(2, "k"))
            ext1[4].append(lambda: xtb_load_t(2, "v"))
            ext1[5].append(lambda: xtb_load_t(2, "q"))
            carry = block(1, ext1, carry)
            gemm4_piece(0, 3, 1)

            # only half of gemm4(1) runs in block 2 (block 2 is PE-bound);
            # the rest fills block 3's ACT-bound slots.
            ext2 = {h: [lambda h=h: gemm4_piece(1, (h - 1) // 2,
                                                (h - 1) % 2)]
                    for h in range(1, 5)}
            ext2[3].append(lambda: xtb_load_t(3, "k"))
            ext2[4].append(lambda: xtb_load_t(3, "v"))
            ext2.setdefault(5, []).append(lambda: xtb_load_t(3, "q"))
            carry = block(2, ext2, carry)

            ext3 = {h: [lambda h=h: gemm4_piece(2, (h - 1) // 2,
                                                (h - 1) % 2)]
                    for h in range(1, 8)}
            for h in range(1, 5):
                ext3[h].append(lambda h=h: gemm4_piece(1, 2 + (h - 1) // 2,
                                                       (h - 1) % 2))
            carry = block(3, ext3, carry)
            gemm4_piece(2, 3, 1)
            # drain: the last two heads' gemm3, then block 3's gemm4
            for pend in carry:
                for f in gemm3_ops(*pend):
                    f()
            for sc in range(4):
                gemm4_piece(3, sc, 0)
                gemm4_piece(3, sc, 1)
    nc.finalize()
    return nc


# ---------------- legacy (dense/no-mask) builder, unchanged ----------------

def build_nc(mode="causal"):
    if mode == "causal":
        return build_nc2()
    raise NotImplementedError("v2 kernel supports the causal mask only")


_CACHE = {}


def _get_nc(mode):
    if mode not in _CACHE:
        _CACHE[mode] = build_nc2() if mode == "causal" else None
    return _CACHE[mode]


def _host_xt(x, bf):
    # xt[p, ss*8192 + dc*512 + si] = x[ss*512+si, dc*128+p]
    xr = np.asarray(x, np.float32).reshape(NSS, 512, NDC, 128)
    return np.ascontiguousarray(
        xr.transpose(3, 0, 2, 1).reshape(128, NDC * S).astype(bf))


def kernel(q, k, v, mask, Wq, bq, Wk, bk, Wv, bv, Wo, bo):
    q = np.asarray(q, np.float32)
    k = np.asarray(k, np.float32)
    v = np.asarray(v, np.float32)
    mask = np.asarray(mask)
    Wq = np.asarray(Wq, np.float32)
    Wk = np.asarray(Wk, np.float32)
    Wv = np.asarray(Wv, np.float32)
    Wo = np.asarray(Wo, np.float32)
    bq = np.asarray(bq, np.float32)
    bk = np.asarray(bk, np.float32)
    bv = np.asarray(bv, np.float32)
    bo = np.asarray(bo, np.float32)

    m = mask.astype(np.float64)
    assert np.array_equal(m, np.tril(np.ones((S, S)))), \
        "v2 kernel supports the causal mask"

    nc = _get_nc("causal")
    bf = ml_dtypes.bfloat16
    tri_np = np.triu(np.ones((128, 128))).astype(bf)
    id_np = np.eye(128).astype(bf)

    head_perm = [h for cc in range(4) for h in (cc, cc + 4)]
    col_perm = np.concatenate(
        [np.arange(h * HD, (h + 1) * HD) for h in head_perm])

    in_maps = []
    for core in range(NCORES):
        b, kb = core // KVSH, core % KVSH
        wq_sh = Wq[:, kb * CQ:(kb + 1) * CQ][:, col_perm]
        wo_sh = Wo[kb * CQ:(kb + 1) * CQ, :][col_perm, :]
        bq_sh = bq[kb * CQ:(kb + 1) * CQ][col_perm]
        wk_sh = Wk[:, kb * CK:(kb + 1) * CK]
        wv_sh = Wv[:, kb * CK:(kb + 1) * CK]
        # cc-major: wq_arr[p, cc*2048 + dc*128 + j]
        wq_arr = wq_sh.reshape(NDC, 128, 4, 128).transpose(1, 2, 0, 3).reshape(
            128, NDC * CQ)
        wkv_arr = np.stack(
            [w.reshape(NDC, 128, CK).transpose(1, 0, 2).reshape(128, NDC * CK)
             for w in (wk_sh, wv_sh)], axis=1).reshape(128, 2 * NDC * CK)
        wo_arr = wo_sh.reshape(4, 128, DIM).transpose(1, 0, 2).reshape(
            128, 4 * DIM)
        im = {
            "qt": _host_xt(q[b], bf),
            "kt": _host_xt(k[b], bf),
            "vt": _host_xt(v[b], bf),
            "wq": np.ascontiguousarray(wq_arr.astype(bf)),
            "wkv": np.ascontiguousarray(wkv_arr.astype(bf)),
            "wo": np.ascontiguousarray(wo_arr.astype(bf)),
            "bq": np.ascontiguousarray(bq_sh),
            "bk": np.ascontiguousarray(bk[kb * CK:(kb + 1) * CK]),
            "bv": np.ascontiguousarray(bv[kb * CK:(kb + 1) * CK]),
            "tri": tri_np,
            "ident": id_np,
        }
        in_maps.append(im)

    res = run_bass_kernel_spmd(nc, in_maps, core_ids=list(range(NCORES)))
    outs = [r["out"] for r in res.results]
    full = np.empty((B, S, DIM), np.float32)
    for b in range(B):
        acc = outs[b * KVSH].astype(np.float32)
        for kb in range(1, KVSH):
            acc = acc + outs[b * KVSH + kb].astype(np.float32)
        full[b] = acc + bo[None, :]
    return full


# revision 57
# speedup vs baseline: 1.0748x; 1.0014x over previous
"""Grouped-query attention (GQA) Trainium2 Bass kernel, v2.

Problem: B=2, S=2048, DIM=2048, HQ=32, HKV=8, HEAD_DIM=64, causal mask.
Sharding: 8 cores = 2 (batch) x 4 (kv-head groups). Core c handles batch
c//4 and kv-block c%4 (2 kv heads, 8 q heads). Wq/Wk/Wv sharded
column-wise, Wo row-wise; each core writes a partial [S, DIM] bf16
output; host sums the 4 partials per batch and adds bo.

v2 changes vs the previous kernel (all bf16; fp8 fails the 2e-2 gate):
  - Host pre-transposes q/k/v (x^T tiles streamed as plain wide DMAs;
    no XBAR dma transpose), DMA order tuned so the GEMM1 chains and the
    first heads' scores are never starved at startup.
  - exp emitted over [128, 1024] PSUM groups: full j-blocks paired, and
    the diagonal tail packed as [512|384|128] + [256] (one exp per
    group; the scores->exp->PSUM-recycle chain paces the pipeline, so
    fewer, wider ACT ops matter).
  - GEMM3 restructured: attn accumulated in natural [i, c] layout with
    exp blocks as the stationary operand and v(+ones) moving -> 65-row
    matmuls at full PE efficiency (~half the PE cycles of the j-layout),
    denominator lands as column 64 per i-partition. Runs two head-slots
    behind the score stream for ACT slack.
  - Normalization fused into the PSUM->SBUF copy: DVE fp32 reciprocal
    of the denominator column + per-partition tensor_scalar multiply.
    The PE broadcast-matmul normalize of v1 is gone.
  - attn^T for GEMM4 via PE transposes of the normalized [128, 64]
    chunks into one [64, 512] PSUM tile, one DVE copy per head back to
    SBUF. GEMM1 bias-adds on DVE; tri masking on GPSIMD (GPSIMD cannot
    touch PSUM); ACT does exp only.
PSUM: 4 banks score-groups (2x[128,1024]) + 2 banks gemm3 accumulators
+ 2 banks shared (gemm1 / vtrans / transposes / gemm4).
"""

import numpy as np
import ml_dtypes

import concourse.bass as bass
import concourse.mybir as mybir
from concourse import bacc
from concourse.tile import TileContext
from concourse.bass_utils import run_bass_kernel_spmd

F32 = mybir.dt.float32
BF16 = mybir.dt.bfloat16
AF = mybir.ActivationFunctionType
ALU = mybir.AluOpType

B, S, DIM = 2, 2048, 2048
HQ, HKV, HD = 32, 8, 64
GROUP = HQ // HKV              # 4
NCORES = 8
KVSH = 4                       # kv-blocks (shards) per batch
CQ = (HQ // KVSH) * HD         # 512 q-proj cols per core (8 heads)
CK = (HKV // KVSH) * HD        # 128 kv-proj cols per core (2 heads)
NDC = DIM // 128               # 16 contraction chunks
NSS = S // 512                 # 4 sequence chunks of 512


def build_nc2():
    """Causal-mode v2 builder."""
    nc = bacc.Bacc("TRN2", target_bir_lowering=False)

    # xt[p, ss*8192 + dc*512 + si] = x[ss*512+si, dc*128+p]
    qt = nc.dram_tensor("qt", [128, NDC * S], BF16, kind="ExternalInput")
    kt = nc.dram_tensor("kt", [128, NDC * S], BF16, kind="ExternalInput")
    vt = nc.dram_tensor("vt", [128, NDC * S], BF16, kind="ExternalInput")
    wq = nc.dram_tensor("wq", [128, NDC * CQ], BF16, kind="ExternalInput")
    wkv = nc.dram_tensor("wkv", [128, 2 * NDC * CK], BF16,
                         kind="ExternalInput")
    wo = nc.dram_tensor("wo", [128, 4 * DIM], BF16, kind="ExternalInput")
    bq = nc.dram_tensor("bq", [CQ], F32, kind="ExternalInput")
    bk = nc.dram_tensor("bk", [CK], F32, kind="ExternalInput")
    bv = nc.dram_tensor("bv", [CK], F32, kind="ExternalInput")
    tri = nc.dram_tensor("tri", [128, 128], BF16, kind="ExternalInput")
    ident = nc.dram_tensor("ident", [128, 128], BF16, kind="ExternalInput")
    out = nc.dram_tensor("out", [S, DIM], BF16, kind="ExternalOutput")

    XTS = {"q": qt, "k": kt, "v": vt}

    with TileContext(nc) as tc:
        with (
            tc.tile_pool(name="consts", bufs=1) as consts,
            tc.tile_pool(name="w", bufs=1) as wpool,
            tc.tile_pool(name="xt", bufs=1) as xt,
            tc.tile_pool(name="acts", bufs=1) as acts,
            tc.tile_pool(name="at2", bufs=3) as at2,
            tc.tile_pool(name="exp", bufs=21) as expp,
            tc.tile_pool(name="exps", bufs=4) as expsp,
            tc.tile_pool(name="nrm", bufs=6) as nrmp,
            tc.tile_pool(name="ob", bufs=5) as obp,
            tc.tile_pool(name="psc", bufs=2, space="PSUM") as psc,
            tc.tile_pool(name="psg", bufs=2, space="PSUM") as psg,
            tc.tile_pool(name="psx", bufs=2, space="PSUM") as psx,
        ):
            cn = {}

            def load_consts_early():
                cn["id"] = consts.tile([128, 128], BF16, tag="id",
                                       name="id_c")
                nc.sync.dma_start(out=cn["id"][:, :], in_=ident[:, :])
                cn["bk"] = consts.tile([128, 1], F32, tag="bk", name="bk_c")
                nc.sync.dma_start(
                    out=cn["bk"][:, :],
                    in_=bass.AP(tensor=bk[0:1].tensor, offset=0,
                                ap=[[1, 128], [128, 1]]))

            def load_consts():
                cn["tri"] = consts.tile([128, 128], BF16, tag="tri",
                                        name="tri_c")
                nc.sync.dma_start(out=cn["tri"][:, :], in_=tri[:, :])
                cn["bq"] = consts.tile([128, 4], F32, tag="bq", name="bq_c")
                nc.sync.dma_start(
                    out=cn["bq"][:, :],
                    in_=bass.AP(tensor=bq[0:1].tensor, offset=0,
                                ap=[[1, 128], [128, 4]]))
                cn["bv"] = consts.tile([128, 128], F32, tag="bv",
                                       name="bv_c")
                nc.sync.dma_start(
                    out=cn["bv"][:, :],
                    in_=bass.AP(tensor=bv[0:1].tensor, offset=0,
                                ap=[[0, 128], [1, 128]]))

            # ---- transposed input loads: plain DMA of host-packed x^T ----
            # Tiles cover [dc_lo, dc_hi) contraction chunks; finer tiles at
            # startup let the first GEMM1 chains begin sooner.
            HDC = NDC // 2
            XTB = {}

            def xtb_piece(ss, nm, dc_lo, dc_hi, tag):
                ndc = dc_hi - dc_lo
                t = xt.tile([128, ndc * 512], BF16, tag=tag,
                            name=f"x{tag}")
                c0 = ss * 8192 + dc_lo * 512
                nc.sync.dma_start(out=t[:, :],
                                  in_=XTS[nm][:, c0:c0 + ndc * 512])
                XTB.setdefault((ss, nm), []).append((dc_lo, dc_hi, t))

            def xtb_half(ss, nm, half):
                xtb_piece(ss, nm, half * HDC, (half + 1) * HDC,
                          f"x{nm}{half}")

            def xtb_load_t(ss, nm):
                xtb_half(ss, nm, 0)
                xtb_half(ss, nm, 1)

            def xtb_load(ss):
                for nm in "kvq":
                    xtb_load_t(ss, nm)

            def xslice(ss, nm, dc):
                for dc_lo, dc_hi, t in XTB[(ss, nm)]:
                    if dc_lo <= dc < dc_hi:
                        return t[:, (dc - dc_lo) * 512:(dc - dc_lo + 1) * 512]
                raise KeyError((ss, nm, dc))

            # ---- weights ----
            # prologue DMA order: k-chain first, then v (vx1 needed by the
            # first gemm3), then q per-cc chunks (cc-major wq layout).
            wk_bf = wpool.tile([128, NDC * CK], BF16, tag="wk", name="wk_bf")
            nc.sync.dma_start(out=wk_bf[:, :], in_=wkv[:, 0:2048])
            load_consts_early()
            for qt_ in range(4):
                xtb_piece(0, "k", qt_ * 4, (qt_ + 1) * 4, f"xk0q{qt_}")
            wv_bf = wpool.tile([128, NDC * CK], BF16, tag="wv", name="wv_bf")
            nc.sync.dma_start(out=wv_bf[:, :], in_=wkv[:, 2048:4096])
            xtb_load_t(0, "v")
            load_consts()
            wqc = []

            def load_wqc(cc):
                wq_c = wpool.tile([128, NDC * 128], BF16, tag=f"wq{cc}",
                                  name=f"wq{cc}")
                nc.sync.dma_start(out=wq_c[:, :],
                                  in_=wq[:, cc * 2048:(cc + 1) * 2048])
                wqc.append(wq_c)

            xtb_half(0, "q", 0)
            load_wqc(0)
            xtb_half(0, "q", 1)
            for cc in range(1, 4):
                load_wqc(cc)
            wo_bf = wpool.tile([128, 4 * DIM], BF16, tag="wo", name="wo_bf")
            nc.sync.dma_start(out=wo_bf[:, :], in_=wo[:, :])

            # ---- persistent activations ----
            qxT = [acts.tile([128, S], BF16, tag=f"qx{cc}", name=f"qx{cc}")
                   for cc in range(4)]
            kxT = acts.tile([128, S], BF16, tag="kx", name="kx")
            vxT = acts.tile([128, S], BF16, tag="vx", name="vx")
            vx1 = [acts.tile([128, 130], BF16, tag=f"vp{sc}", name=f"vp{sc}")
                   for sc in range(S // 128)]

            def attnT(ss, cc):
                # double-buffered across ss (gemm4 runs one block behind)
                return at2.tile([128, 512], BF16, tag=f"at{cc}",
                                name=f"at{ss}{cc}")

            attnTs = {}

            def gemm1_q_cc(ss, cc):
                s0 = ss * 512
                ps = psx.tile([128, 512], F32, tag="m")
                for dc in range(NDC):
                    nc.tensor.matmul(
                        ps[:, :],
                        wqc[cc][:, dc * 128:(dc + 1) * 128],
                        xslice(ss, "q", dc),
                        start=(dc == 0), stop=(dc == NDC - 1))
                nc.vector.tensor_scalar_add(qxT[cc][:, s0:s0 + 512],
                                            ps[:, :], cn["bq"][:, cc:cc + 1])

            def gemm1_k(ss):
                s0 = ss * 512
                ps = psx.tile([128, 512], F32, tag="m")
                for dc in range(NDC):
                    nc.tensor.matmul(
                        ps[:, :], wk_bf[:, dc * 128:(dc + 1) * 128],
                        xslice(ss, "k", dc),
                        start=(dc == 0), stop=(dc == NDC - 1))
                nc.vector.tensor_scalar_add(kxT[:, s0:s0 + 512], ps[:, :],
                                            cn["bk"][:, 0:1])

            def gemm1_v(ss):
                s0 = ss * 512
                ps = psx.tile([128, 512], F32, tag="m")
                for dc in range(NDC):
                    nc.tensor.matmul(
                        ps[:, :], wv_bf[:, dc * 128:(dc + 1) * 128],
                        xslice(ss, "v", dc),
                        start=(dc == 0), stop=(dc == NDC - 1))
                nc.vector.tensor_copy(vxT[:, s0:s0 + 512], ps[:, :])

            def vtrans(ss):
                s0 = ss * 512
                vtp = psx.tile([128, 512], BF16, tag="m")
                for sc in range(4):
                    nc.tensor.transpose(
                        vtp[:, sc * 128:(sc + 1) * 128],
                        vxT[:, s0 + sc * 128:s0 + (sc + 1) * 128],
                        cn["id"][:, :])
                for sc in range(4):
                    jb = ss * 4 + sc
                    vx = vx1[jb]
                    for h2 in range(2):
                        nc.vector.tensor_tensor(
                            vx[:, h2 * 65:h2 * 65 + 64],
                            vtp[:, sc * 128 + h2 * 64:sc * 128 + (h2 + 1) * 64],
                            cn["bv"][:, h2 * 64:(h2 + 1) * 64], ALU.add)
                    nc.vector.memset(vx[:, 64:65], 1.0)
                    nc.vector.memset(vx[:, 129:130], 1.0)

            # exinfo[(ss, h, jb)] = (sbuf exp tile, col0, off)
            exinfo = {}

            def scores_grp(ss, h, grp):
                """One PSUM tile holding the given (jb, col0) j-blocks
                packed back-to-back: matmuls + a single exp (+ tri for
                diagonal blocks)."""
                s0 = ss * 512
                th, po, kv = h % 4, (h // GROUP) * 64, h // GROUP
                tot = max(c0 + 512 - max(0, jb * 128 - s0)
                          for jb, c0 in grp)
                sp = psc.tile([128, 1024], F32, tag="sc")
                if tot > 512:
                    ex = expp.tile([128, 1024], BF16, tag="exp")
                else:
                    ex = expsp.tile([128, 512], BF16, tag="exps")
                for jb, c0 in grp:
                    j0 = jb * 128
                    off = max(0, j0 - s0)
                    N = 512 - off
                    nc.tensor.matmul(
                        sp[:, c0:c0 + N],
                        kxT[kv * 64:(kv + 1) * 64, j0:j0 + 128],
                        qxT[th][po:po + 64, s0 + off:s0 + 512],
                        start=True, stop=True)
                    exinfo[(ss, h, jb)] = (ex, c0, off)
                nc.scalar.activation(ex[:, :tot], sp[:, :tot], AF.Exp,
                                     scale=0.125)
                for jb, c0 in grp:
                    if jb >= 4 * ss:
                        nc.gpsimd.tensor_tensor(
                            ex[:, c0:c0 + 128], ex[:, c0:c0 + 128],
                            cn["tri"][:, :], ALU.mult)

            def scores_plan(ss):
                """Full blocks paired; the diagonal tail packed as
                [512|384|128] (exactly one [128,1024] tile) + [256]."""
                d = 4 * ss
                grps = [[(jb, 0), (jb + 1, 512)] for jb in range(0, d, 2)]
                grps.append([(d, 0), (d + 1, 512), (d + 3, 896)])
                grps.append([(d + 2, 0)])
                return grps

            def gemm3_chunk(ss, h, sc):
                """attn chunk [128 i, 65] for i-block ib = 4ss+sc; returns
                psum tile."""
                kv = h // GROUP
                ib = 4 * ss + sc
                at = psg.tile([128, 512], F32, tag="g3")
                for jb in range(ib + 1):
                    ex, c0, off = exinfo[(ss, h, jb)]
                    nc.tensor.matmul(
                        at[:, 0:65],
                        ex[:, c0 + sc * 128 - off:c0 + sc * 128 - off + 128],
                        vx1[jb][:, kv * 65:kv * 65 + 65],
                        start=(jb == 0), stop=(jb == ib))
                return at

            def gemm3_norm(ss, h, sc, at):
                """fp32 reciprocal + fused normalize into SBUF copy."""
                rcp = nrmp.tile([128, 1], F32, tag="rcp")
                nc.vector.reciprocal(rcp[:, :], at[:, 64:65])
                an = nrmp.tile([128, 64], BF16, tag="an")
                nc.vector.tensor_scalar_mul(an[:, :], at[:, 0:64], rcp[:, :])
                return an

            def gemm3_ops(ss, h):
                """5 closures: chunk+norm x4 with transposes delayed so
                the DVE norm is long done, then one [64, 512] copy into
                attnT[ss]."""
                th, po = h % 4, (h // GROUP) * 64
                st = {}

                def chunk(sc):
                    def f():
                        at = gemm3_chunk(ss, h, sc)
                        st[sc] = gemm3_norm(ss, h, sc, at)
                        if sc == 2:
                            st["tr"] = psx.tile([64, 512], BF16, tag="m",
                                                name="tr")
                            for lo in (0, 1):
                                nc.tensor.transpose(
                                    st["tr"][0:64, lo * 128:(lo + 1) * 128],
                                    st[lo][:, :], cn["id"][:, :])
                        elif sc == 3:
                            nc.tensor.transpose(
                                st["tr"][0:64, 256:384],
                                st[2][:, :], cn["id"][:, :])
                    return f

                def fin():
                    nc.tensor.transpose(st["tr"][0:64, 384:512],
                                        st[3][:, :], cn["id"][:, :])
                    nc.vector.tensor_copy(
                        attnTs[(ss, th)][po:po + 64, :], st["tr"][0:64, :])

                return [chunk(0), chunk(1), chunk(2), chunk(3), fin]

            def gemm4_piece(ss, sc, eh):
                s0 = ss * 512
                i0 = s0 + sc * 128
                ob = obp.tile([128, 1024], BF16, tag="ob")
                for e2 in range(2):
                    ec = eh * 2 + e2
                    g4 = psx.tile([128, 512], F32, tag="m")
                    for cc2 in range(4):
                        nc.tensor.matmul(
                            g4[:, :],
                            attnTs[(ss, cc2)][:, sc * 128:(sc + 1) * 128],
                            wo_bf[:, cc2 * 2048 + ec * 512:
                                  cc2 * 2048 + (ec + 1) * 512],
                            start=(cc2 == 0), stop=(cc2 == 3))
                    nc.vector.tensor_copy(
                        ob[:, e2 * 512:(e2 + 1) * 512], g4[:, :])
                    nc.sync.dma_start(
                        out=out[i0:i0 + 128,
                                eh * 1024 + e2 * 512:eh * 1024 + (e2 + 1) * 512],
                        in_=ob[:, e2 * 512:(e2 + 1) * 512])

            def block(ss, extra, carry_in):
                """scores(h) + gemm3(h-1) pipeline; carry_in = (pss, 7) of
                the previous block's last head, processed at h==0."""
                for cc in range(4):
                    attnTs[(ss, cc)] = attnT(ss, cc)
                if ss == 0:
                    gemm1_k(ss)
                    gemm1_v(ss)
                    vtrans(ss)
                    gemm1_q_cc(ss, 0)
                else:
                    gemm1_k(ss)
                    gemm1_v(ss)
                grps = scores_plan(ss)
                pending = list(carry_in or [])
                for h in range(8):
                    # q0 must precede this slot's scores; the rest of the
                    # projection work rides the fill list so the carried
                    # gemm3's DVE norm ops are served first (psg recycle).
                    tail_ops = []
                    if h == 0 and ss > 0:
                        gemm1_q_cc(ss, 0)
                        tail_ops = [lambda: vtrans(ss),
                                    lambda: gemm1_q_cc(ss, 1)]
                    elif h == 0:
                        tail_ops = [lambda: gemm1_q_cc(ss, 1)]
                    elif h == 1:
                        tail_ops = [lambda: gemm1_q_cc(ss, 2)]
                    elif h == 2:
                        tail_ops = [lambda: gemm1_q_cc(ss, 3)]
                    # interleave: scores groups of head h with gemm3 of the
                    # head TWO slots back (extra ACT slack) and this slot's
                    # extra ops (gemm4 etc.), fills front-loaded.
                    ng = len(grps)
                    g3ops = []
                    if len(pending) >= 2:
                        g3ops = gemm3_ops(*pending.pop(0))
                    fill = list(g3ops) + tail_ops + list(extra.get(h, ()))
                    n_emit, n_tot = 0, len(fill)
                    for gi, grp in enumerate(grps):
                        scores_grp(ss, h, grp)
                        want = min(n_tot, -(-n_tot * (gi + 1) // ng))
                        while n_emit < want:
                            fill[n_emit]()
                            n_emit += 1
                    while n_emit < n_tot:
                        fill[n_emit]()
                        n_emit += 1
                    pending.append((ss, h))
                return pending

            # ---- schedule ----
            xtb_load(1)
            carry = block(0, {}, None)

            ext1 = {h: [lambda h=h: gemm4_piece(0, (h - 1) // 2,
                                                (h - 1) % 2)]
                    for h in range(1, 8)}
            ext1[3].append(lambda: xtb_load_t(2, "k"))
            ext1[4].append(lambda: xtb_load_t(2, "v"))
            ext1[5].append(lambda: xtb_load_t(2, "q"))
            carry = block(1, ext1, carry)
            gemm4_piece(0, 3, 1)

            # only half of gemm4(1) runs in block 2 (block 2 is PE-bound);
            # the rest fills block 3's ACT-bound slots.
            ext2 = {h: [lambda h=h: gemm4_piece(1, (h - 1) // 2,
                                                (h - 1) % 2)]
                    for h in range(1, 5)}
            ext2[3].append(lambda: xtb_load_t(3, "k"))
            ext2[4].append(lambda: xtb_load_t(3, "v"))
            ext2.setdefault(5, []).append(lambda: xtb_load_t(3, "q"))
            carry = block(2, ext2, carry)

            ext3 = {h: [lambda h=h: gemm4_piece(2, (h - 1) // 2,
                                                (h - 1) % 2)]
                    for h in range(1, 8)}
            for h in range(1, 5):
                ext3[h].append(lambda h=h: gemm4_piece(1, 2 + (h - 1) // 2,
                                                       (h - 1) % 2))
            carry = block(3, ext3, carry)
            gemm4_piece(2, 3, 1)
            # drain: the last two heads' gemm3, then block 3's gemm4
            for pend in carry:
                for f in gemm3_ops(*pend):
                    f()
            for sc in range(4):
                gemm4_piece(3, sc, 0)
                gemm4_piece(3, sc, 1)
    nc.finalize()
    return nc


# ---------------- legacy (dense/no-mask) builder, unchanged ----------------

def build_nc(mode="causal"):
    if mode == "causal":
        return build_nc2()
    raise NotImplementedError("v2 kernel supports the causal mask only")


_CACHE = {}


def _get_nc(mode):
    if mode not in _CACHE:
        _CACHE[mode] = build_nc2() if mode == "causal" else None
    return _CACHE[mode]


def _host_xt(x, bf):
    # xt[p, ss*8192 + dc*512 + si] = x[ss*512+si, dc*128+p]
    xr = np.asarray(x, np.float32).reshape(NSS, 512, NDC, 128)
    return np.ascontiguousarray(
        xr.transpose(3, 0, 2, 1).reshape(128, NDC * S).astype(bf))


def kernel(q, k, v, mask, Wq, bq, Wk, bk, Wv, bv, Wo, bo):
    q = np.asarray(q, np.float32)
    k = np.asarray(k, np.float32)
    v = np.asarray(v, np.float32)
    mask = np.asarray(mask)
    Wq = np.asarray(Wq, np.float32)
    Wk = np.asarray(Wk, np.float32)
    Wv = np.asarray(Wv, np.float32)
    Wo = np.asarray(Wo, np.float32)
    bq = np.asarray(bq, np.float32)
    bk = np.asarray(bk, np.float32)
    bv = np.asarray(bv, np.float32)
    bo = np.asarray(bo, np.float32)

    m = mask.astype(np.float64)
    assert np.array_equal(m, np.tril(np.ones((S, S)))), \
        "v2 kernel supports the causal mask"

    nc = _get_nc("causal")
    bf = ml_dtypes.bfloat16
    tri_np = np.triu(np.ones((128, 128))).astype(bf)
    id_np = np.eye(128).astype(bf)

    head_perm = [h for cc in range(4) for h in (cc, cc + 4)]
    col_perm = np.concatenate(
        [np.arange(h * HD, (h + 1) * HD) for h in head_perm])

    in_maps = []
    for core in range(NCORES):
        b, kb = core // KVSH, core % KVSH
        wq_sh = Wq[:, kb * CQ:(kb + 1) * CQ][:, col_perm]
        wo_sh = Wo[kb * CQ:(kb + 1) * CQ, :][col_perm, :]
        bq_sh = bq[kb * CQ:(kb + 1) * CQ][col_perm]
        wk_sh = Wk[:, kb * CK:(kb + 1) * CK]
        wv_sh = Wv[:, kb * CK:(kb + 1) * CK]
        # cc-major: wq_arr[p, cc*2048 + dc*128 + j]
        wq_arr = wq_sh.reshape(NDC, 128, 4, 128).transpose(1, 2, 0, 3).reshape(
            128, NDC * CQ)
        wkv_arr = np.stack(
            [w.reshape(NDC, 128, CK).transpose(1, 0, 2).reshape(128, NDC * CK)
             for w in (wk_sh, wv_sh)], axis=1).reshape(128, 2 * NDC * CK)
        wo_arr = wo_sh.reshape(4, 128, DIM).transpose(1, 0, 2).reshape(
            128, 4 * DIM)
        im = {
            "qt": _host_xt(q[b], bf),
            "kt": _host_xt(k[b], bf),
            "vt": _host_xt(v[b], bf),
            "wq": np.ascontiguousarray(wq_arr.astype(bf)),
            "wkv": np.ascontiguousarray(wkv_arr.astype(bf)),
            "wo": np.ascontiguousarray(wo_arr.astype(bf)),
            "bq": np.ascontiguousarray(bq_sh),
            "bk": np.ascontiguousarray(bk[kb * CK:(kb + 1) * CK]),
            "bv": np.ascontiguousarray(bv[kb * CK:(kb + 1) * CK]),
            "tri": tri_np,
            "ident": id_np,
        }
        in_maps.append(im)

    res = run_bass_kernel_spmd(nc, in_maps, core_ids=list(range(NCORES)))
    outs = [r["out"] for r in res.results]
    full = np.empty((B, S, DIM), np.float32)
    for b in range(B):
        acc = outs[b * KVSH].astype(np.float32)
        for kb in range(1, KVSH):
            acc = acc + outs[b * KVSH + kb].astype(np.float32)
        full[b] = acc + bo[None, :]
    return full
